# revision 27
# baseline (speedup 1.0000x reference)
"""MicroGPT forward pass on 8 Trainium2 NeuronCores (Bass/Tile).

Sharding: token-sharded — core c = 2*b + h owns batch b, sequence half h
(512 contiguous tokens). Activations are feature-major in SBUF
(x^T: [768 rows -> 6 tiles of 128, 512 token cols]); all matmuls fp32r/f16.
Attention: S^T = K^T-slice (stationary) x Q^T (moving); softmax without max
subtraction (scores bounded); denominators via a ones column appended to V.
K/V slots 0-3 are the core's own 4 blocks (block-causal masks, identical on
every core); slots 4-7 are the pair core's blocks, weighted by a per-core
0/1 scalar (1 when the pair holds earlier positions). Per layer a PAIRWISE
(2-rank) AllGather shares K^T/V; each core reads its pair's shard with a
register-indexed dynamic DMA. All per-token statistic broadcasts (LN
rstd/mean, softmax reciprocal) are done with K=1 matmuls on the PE instead
of DMA round-trips. Gelu is a single fused ACT op; the FFN runs fc1 for all
24 f-tiles first, then fc2 m-outer with all fc2 weights SBUF-resident so
the PE streams uninterrupted. Final token: masked AllReduce, then final LN +
vocab-sharded unembed (4000 vocab rows per core, preloaded to SBUF).
"""
import sys, math

sys.path.insert(0, "/opt/trn_rl_repo")
import numpy as np

import concourse.bass as bass
import concourse.bacc as bacc
import concourse.mybir as mybir
import concourse.tile as tile

D, NH, DH, FF, NL, V = 768, 12, 64, 3072, 4, 32000
B, S = 4, 1024
EPS = 1e-5
NC_ = 8
P = 128
T = 512            # tokens per core
DT = D // P        # 6 d-model tiles
FT = FF // P       # 24 ff tiles
KB = 8             # key slots (0-3 own, 4-7 pair)
VS = V // NC_      # 4000 vocab rows per core
VCH = 8            # vocab chunks of 500
VCW = VS // VCH    # 500
F32 = mybir.dt.float32
F32R = mybir.dt.float32r
F16 = mybir.dt.float16
BF16 = mybir.dt.bfloat16
I32 = mybir.dt.int32
AF = mybir.ActivationFunctionType
OP = mybir.AluOpType
SCALE = 1.0 / math.sqrt(DH)
VW = NH * (DH + 1)           # 780 — V tile width incl. ones cols
CONTRIB_W = DT * T + 4 * VW  # AllGather contribution width


# ---------------------------------------------------------------- bass program
def build_nc(n_layers=NL, pcol=511, dbg=False):
    nc = bacc.Bacc(None, target_bir_lowering=False, debug=False, num_devices=NC_)

    x0T = nc.dram_tensor("x0T", [DT, P, T], F32R, kind="ExternalInput")
    wqT = nc.dram_tensor("wqT", [n_layers, P, DT * D], F16, kind="ExternalInput")
    wkT = nc.dram_tensor("wkT", [n_layers, P, DT * D], F16, kind="ExternalInput")
    wvT = nc.dram_tensor("wvT", [n_layers, P, DT * D], F16, kind="ExternalInput")
    woT = nc.dram_tensor("woT", [n_layers, P, DT * D], F16, kind="ExternalInput")
    fc1T = nc.dram_tensor("fc1T", [n_layers, FT, P, DT * P], F16, kind="ExternalInput")
    fc2T = nc.dram_tensor("fc2T", [n_layers, FT, P, D], F16, kind="ExternalInput")
    ln1g = nc.dram_tensor("ln1g", [n_layers, P, DT], F32, kind="ExternalInput")
    ln1b = nc.dram_tensor("ln1b", [n_layers, P, DT], F32, kind="ExternalInput")
    ln2g = nc.dram_tensor("ln2g", [n_layers, P, DT], F32, kind="ExternalInput")
    ln2b = nc.dram_tensor("ln2b", [n_layers, P, DT], F32, kind="ExternalInput")
    lnfg = nc.dram_tensor("lnfg", [P, DT], F32, kind="ExternalInput")
    lnfb = nc.dram_tensor("lnfb", [P, DT], F32, kind="ExternalInput")
    uT = nc.dram_tensor("uT", [DT, P, VS], F16, kind="ExternalInput")
    masks = nc.dram_tensor("masks", [P, P], F16, kind="ExternalInput")
    remw = nc.dram_tensor("remw", [P, 1], F32, kind="ExternalInput")
    sel4 = nc.dram_tensor("sel4", [P, B], F32R, kind="ExternalInput")
    pairsel = nc.dram_tensor("pairsel", [1, 1], I32, kind="ExternalInput")

    out = nc.dram_tensor("out", [B, VS], F32, kind="ExternalOutput")

    from contextlib import ExitStack
    with tile.TileContext(nc) as tc:
        with ExitStack() as _stk:
            _p = lambda *a, **kw: _stk.enter_context(tc.tile_pool(*a, **kw))
            cpool = _p(name="const", bufs=1)
            ppool = _p(name="persist", bufs=1)
            xpool = _p(name="xp", bufs=6)
            hpool = _p(name="hp", bufs=6)
            qpool = _p(name="qp", bufs=6)
            apool = _p(name="ac", bufs=6)
            wpool = _p(name="wp", bufs=3)
            fpool = _p(name="fp", bufs=3)
            f2pool = _p(name="f2p", bufs=FT)
            gpool = _p(name="gp", bufs=FT)
            upool = _p(name="up", bufs=4)
            epool = _p(name="ep", bufs=4)
            spool = _p(name="sp", bufs=5)
            psr = _p(name="psr", bufs=2, space="PSUM")
            psa = _p(name="psa", bufs=2, space="PSUM")
            psb = _p(name="psb", bufs=2, space="PSUM")
            dpool = _p(name="dram", bufs=2, space="DRAM")
            # ---- constants (memset cannot write f32r; stage via f32 + copy)
            ones_f32 = cpool.tile([P, 1], F32)
            nc.vector.memset(ones_f32[:], 1.0)
            trimask = cpool.tile([P, P], F16)
            nc.sync.dma_start(trimask[:], masks[:])
            ones_col = cpool.tile([P, 1], F32R)
            nc.vector.tensor_copy(ones_col[:], ones_f32[:])
            onesr_f32 = cpool.tile([1, P], F32)
            nc.vector.memset(onesr_f32[:], 1.0)
            ones_row = cpool.tile([1, P], F32R)
            nc.vector.tensor_copy(ones_row[:], onesr_f32[:])
            eps1 = cpool.tile([1, 1], F32)
            nc.vector.memset(eps1[:], EPS)
            hse_f = cpool.tile([1, P], F32)
            nc.vector.memset(hse_f[:], 0.0)
            nc.vector.memset(hse_f[0:1, 0:DH], 1.0)
            hsel_e = cpool.tile([1, P], F32R)
            nc.vector.tensor_copy(hsel_e[:], hse_f[:])
            hso_f = cpool.tile([1, P], F32)
            nc.vector.memset(hso_f[:], 0.0)
            nc.vector.memset(hso_f[0:1, DH:P], 1.0)
            hsel_o = cpool.tile([1, P], F32R)
            nc.vector.tensor_copy(hsel_o[:], hso_f[:])
            sel4_sb = cpool.tile([P, B], F32R)
            nc.sync.dma_start(sel4_sb[:], sel4[:])
            remw_sb = cpool.tile([P, 1], F32)
            nc.sync.dma_start(remw_sb[:], remw[:])

            # persistent K^T / V buffers (slots 0-3 own, 4-7 pair)
            KT = [ppool.tile([P, KB * P], F16, tag=f"kt{e}", name=f"KT{e}")
                  for e in range(DT)]
            VT = [ppool.tile([P, VW], F16, tag=f"vt{j}", name=f"VT{j}")
                  for j in range(KB)]
            for j in range(4):
                for h in range(NH):
                    nc.vector.tensor_copy(
                        VT[j][:, h * (DH + 1) + DH : h * (DH + 1) + DH + 1],
                        ones_f32[:])

            # pair rank register for dynamic reads of the AllGather output
            with tc.tile_critical():
                with nc.sync.register("pairreg") as preg:
                    nc.sync.reg_load(preg, pairsel[0:1, 0:1])
                    pv = nc.sync.snap(preg, min_val=0, max_val=1)

            # ---- residual stream (updated in place by residual adds)
            xT = []
            for k in range(DT):
                t_ = xpool.tile([P, T], F32R, tag="xT", name=f"xT{k}")
                nc.sync.dma_start(t_[:], x0T[k])
                xT.append(t_)

            def layer_norm(g_dram, b_dram, l):
                """mean/var via PE stats matmuls; rstd & -mean*rstd broadcast
                to all partitions with a K=1 matmul (no DMA round-trip)."""
                gb = spool.tile([P, 2 * DT], F32, tag="lngb", bufs=3, name="gb")
                nc.sync.dma_start(gb[:, 0:DT], g_dram[l])
                nc.sync.dma_start(gb[:, DT : 2 * DT], b_dram[l])
                sum_ps = psa.tile([1, T], F32, tag="acc", space="PSUM", name="sum_ps")
                sq_ps = psa.tile([1, T], F32, tag="acc", space="PSUM", name="sq_ps")
                sum_ps, sq_ps = sum_ps[:], sq_ps[:]
                for k in range(DT):
                    xsq = epool.tile([P, T], F32R, tag="lntmp", name="xsq")
                    nc.vector.tensor_mul(xsq[:], xT[k][:], xT[k][:])
                    nc.tensor.matmul(sum_ps, ones_col[:], xT[k][:],
                                     start=(k == 0), stop=(k == DT - 1))
                    nc.tensor.matmul(sq_ps, ones_col[:], xsq[:],
                                     start=(k == 0), stop=(k == DT - 1))
                sums_sb = spool.tile([1, T], F32, tag="lnstat", bufs=4, name="sums_sb")
                nc.vector.tensor_copy(sums_sb[:], sum_ps)
                m2s = spool.tile([1, T], F32, tag="lnstat", bufs=4, name="m2s")
                nc.vector.scalar_tensor_tensor(out=m2s[:], in0=sums_sb[:],
                                               scalar=1.0 / (D * D), in1=sums_sb[:],
                                               op0=OP.mult, op1=OP.mult)
                var = spool.tile([1, T], F32, tag="lnstat", bufs=4, name="var")
                nc.vector.scalar_tensor_tensor(out=var[:], in0=sq_ps,
                                               scalar=1.0 / D, in1=m2s[:],
                                               op0=OP.mult, op1=OP.subtract)
                std = spool.tile([1, T], F32, tag="lnstat", bufs=4, name="std")
                nc.scalar.activation(std[:], var[:], AF.Sqrt, bias=eps1[:])
                # rm = [rstd | -mean*rstd] in one f32r row
                rstd = spool.tile([1, T], F32, tag="lnstat", bufs=4, name="rstd")
                nc.vector.reciprocal_approx_fast(out=rstd[:], in_=std[:])
                rm = spool.tile([1, 2 * T], F32R, tag="lnr", bufs=2, name="rm")
                nc.vector.tensor_copy(rm[0:1, 0:T], rstd[:])
                nc.vector.scalar_tensor_tensor(out=rm[0:1, T : 2 * T],
                                               in0=sums_sb[:], scalar=-1.0 / D,
                                               in1=rstd[:],
                                               op0=OP.mult, op1=OP.mult)
                bc_ps = psb.tile([P, 2 * T], F32, tag="bc", space="PSUM",
                                 name="bc_ps")
                nc.tensor.matmul(bc_ps[:, 0:T], ones_row[:],
                                 rm[0:1, 0:T], start=True, stop=True)
                nc.tensor.matmul(bc_ps[:, T : 2 * T], ones_row[:],
                                 rm[0:1, T : 2 * T], start=True, stop=True)
                hT = []
                for k in range(DT):
                    t1 = epool.tile([P, T], F32, tag="lntmp", name="lnt1")
                    nc.vector.tensor_mul(t1[:], bc_ps[:, 0:T], xT[k][:])
                    t2 = epool.tile([P, T], F32, tag="lntmp", name="lnt2")
                    nc.vector.tensor_add(t2[:], bc_ps[:, T : 2 * T], t1[:])
                    h_ = hpool.tile([P, T], F16, tag="hT", name="hT_t")
                    nc.scalar.activation(h_[:], t2[:], AF.Identity,
                                         scale=gb[:, k : k + 1],
                                         bias=gb[:, DT + k : DT + k + 1])
                    hT.append(h_)
                return hT

            for l in range(n_layers):
                with nc.named_scope(f"L{l}"):
                    hT = layer_norm(ln1g, ln1b, l)

                    # ---- K^T, V first (feeds AllGather early), then Q^T
                    wk_sb = wpool.tile([P, DT * D], F16, tag="w", name="wk_sb")
                    nc.sync.dma_start(wk_sb[:], wkT[l])
                    for m in range(DT):
                        k_ps = psr.tile([P, T], F32, tag="rot", space="PSUM", name="k_ps")
                        for k in range(DT):
                            nc.tensor.matmul(
                                k_ps[:], wk_sb[:, k * D + m * P : k * D + (m + 1) * P],
                                hT[k][:], start=(k == 0), stop=(k == DT - 1))
                        nc.vector.tensor_copy(KT[m][:, 0:T], k_ps[:])

                    wv_sb = wpool.tile([P, DT * D], F16, tag="w", name="wv_sb")
                    nc.sync.dma_start(wv_sb[:], wvT[l])
                    for m in range(4):
                        for c in range(2):
                            v_ps = psr.tile([P, 6 * DH], F32, tag="rot", space="PSUM",
                                            name="v_ps")
                            for k in range(DT):
                                nc.tensor.matmul(
                                    v_ps[:], hT[k][:, m * P : (m + 1) * P],
                                    wv_sb[:, k * D + c * 6 * DH : k * D + (c + 1) * 6 * DH],
                                    start=(k == 0), stop=(k == DT - 1))
                            dst = VT[m][:, c * 6 * (DH + 1) : (c + 1) * 6 * (DH + 1)] \
                                .rearrange("p (h e) -> p h e", h=6, e=DH + 1)[:, :, 0:DH]
                            src = v_ps[:].rearrange("p (h e) -> p h e", h=6, e=DH)
                            nc.vector.tensor_copy(dst, src)

                    # ---- share K^T/V with the pair core (2-rank AllGather)
                    contrib = dpool.tile([P, CONTRIB_W], F16, tag="contrib", name="contrib")
                    for e in range(DT):
                        nc.sync.dma_start(contrib[:, e * T : (e + 1) * T],
                                          KT[e][:, 0:T])
                    for m in range(4):
                        nc.sync.dma_start(
                            contrib[:, DT * T + m * VW : DT * T + (m + 1) * VW],
                            VT[m][:])
                    gout = dpool.tile([2, P, CONTRIB_W], F16, tag="gout",
                                      name="gout")
                    nc.gpsimd.collective_compute(
                        "AllGather", OP.bypass,
                        ins=[contrib[:].opt()],
                        outs=[gout[:].opt()],
                        replica_groups=[[2 * i, 2 * i + 1] for i in range(4)],
                    )
                    rsrc = gout[bass.ds(pv, 1)]
                    for e in range(DT):
                        nc.sync.dma_start(KT[e][:, T : 2 * T],
                                          rsrc[0, :, e * T : (e + 1) * T])
                    for m in range(4):
                        nc.sync.dma_start(
                            VT[4 + m][:],
                            rsrc[0, :, DT * T + m * VW : DT * T + (m + 1) * VW])
                        nc.vector.tensor_scalar_mul(VT[4 + m][:], VT[4 + m][:],
                                                    remw_sb[:, 0:1])

                    # prefetch fc2 weights during attention
                    f2ws = []
                    for f in range(FT):
                        f2w = f2pool.tile([P, D], F16, tag="f2w", name="f2w")
                        nc.sync.dma_start(f2w[:], fc2T[l, f])
                        f2ws.append(f2w)

                    wq_sb = wpool.tile([P, DT * D], F16, tag="w", name="wq_sb")
                    nc.sync.dma_start(wq_sb[:], wqT[l])
                    QT = []
                    for m in range(DT):
                        q_ps = psr.tile([P, T], F32, tag="rot", space="PSUM", name="q_ps")
                        for k in range(DT):
                            nc.tensor.matmul(
                                q_ps[:], wq_sb[:, k * D + m * P : k * D + (m + 1) * P],
                                hT[k][:], start=(k == 0), stop=(k == DT - 1))
                        qt = qpool.tile([P, T], F16, tag="qt", name="qt")
                        nc.vector.tensor_copy(qt[:], q_ps[:])
                        QT.append(qt)

                    # ---- attention, head pairs (2 heads share one attnC tile)
                    # own slots j<4: only queries >= slot start (suffix);
                    # remote slots: full width, merged j={4,5} / {6,7} into one
                    # [P, 2T] score tile -> single exp; V zeroed on h=0 cores.
                    attnC = [apool.tile([P, T], F16, tag="attnC", name=f"attnC{e}")
                             for e in range(DT)]
                    # phase 1: OWN slots for every pair (no AllGather dependency
                    # -> runs during the collective); partial numerators land in
                    # attnC, denominators at partitions 0/64 of a small tile.
                    denO = []
                    for e in range(DT):
                        h0, h1 = 2 * e, 2 * e + 1
                        attn_ps = {
                            h0: psa.tile([DH + 1, T], F32, tag="acc",
                                         space="PSUM", name=f"attnps{h0}"),
                            h1: psa.tile([DH + 1, T], F32, tag="acc",
                                         space="PSUM", name=f"attnps{h1}"),
                        }
                        for j in range(4):
                            c0 = j * P
                            N = T - c0
                            sps = {}
                            for h in (h0, h1):
                                base = (h % 2) * DH
                                s_ps = psr.tile([P, T], F32, tag="rot",
                                                space="PSUM", name="s_ps")
                                nc.tensor.matmul(
                                    s_ps[:, 0:N],
                                    KT[e][base : base + DH, j * P : (j + 1) * P],
                                    QT[e][base : base + DH, c0:T],
                                    start=True, stop=True)
                                sps[h] = s_ps
                            es = {}
                            for h in (h0, h1):
                                e_sb = epool.tile([P, T], F16, tag="e", name="e_sb")
                                nc.scalar.activation(e_sb[:, 0:N], sps[h][:, 0:N],
                                                     AF.Exp, scale=SCALE)
                                nc.vector.tensor_mul(e_sb[:, 0:P], e_sb[:, 0:P],
                                                     trimask[:])
                                es[h] = e_sb
                            for h in (h0, h1):
                                nc.tensor.matmul(
                                    attn_ps[h][:, c0:T],
                                    VT[j][:, h * (DH + 1) : (h + 1) * (DH + 1)],
                                    es[h][:, 0:N],
                                    start=(j == 0), stop=(j == 3))
                        dp = spool.tile([DH + 1, T], F16, tag="deno", bufs=6,
                                        name="dp")
                        for h in (h0, h1):
                            base = (h % 2) * DH
                            nc.vector.tensor_copy(attnC[e][base : base + DH, :],
                                                  attn_ps[h][0:DH, :])
                            nc.vector.tensor_copy(dp[base : base + 1, :],
                                                  attn_ps[h][DH : DH + 1, :])
                        denO.append(dp)
                    # phase 2: REMOTE slots (pair data) + combine + normalize
                    for e in range(DT):
                        h0, h1 = 2 * e, 2 * e + 1
                        attn_ps = {
                            h0: psa.tile([DH + 1, T], F32, tag="acc",
                                         space="PSUM", name=f"attnpr{h0}"),
                            h1: psa.tile([DH + 1, T], F32, tag="acc",
                                         space="PSUM", name=f"attnpr{h1}"),
                        }
                        for jp in (4, 6):
                            sws = {}
                            for h in (h0, h1):
                                sws[h] = psb.tile([P, 2 * T], F32, tag="bc",
                                                  space="PSUM", name="sw")
                            for jo in (0, 1):
                                for h in (h0, h1):
                                    base = (h % 2) * DH
                                    nc.tensor.matmul(
                                        sws[h][:, jo * T : (jo + 1) * T],
                                        KT[e][base : base + DH,
                                              (jp + jo) * P : (jp + jo + 1) * P],
                                        QT[e][base : base + DH, :],
                                        start=True, stop=True)
                            ews = {}
                            for h in (h0, h1):
                                ew = epool.tile([P, 2 * T], F16, tag="e", name="ew")
                                nc.scalar.activation(ew[:], sws[h][:], AF.Exp,
                                                     scale=SCALE)
                                ews[h] = ew
                            for h in (h0, h1):
                                nc.tensor.matmul(
                                    attn_ps[h][:],
                                    VT[jp][:, h * (DH + 1) : (h + 1) * (DH + 1)],
                                    ews[h][:, 0:T],
                                    start=(jp == 4), stop=False)
                                nc.tensor.matmul(
                                    attn_ps[h][:],
                                    VT[jp + 1][:, h * (DH + 1) : (h + 1) * (DH + 1)],
                                    ews[h][:, T : 2 * T],
                                    start=False, stop=(jp == 6))
                        # drain accumulators first (denominator read + combine
                        # add) so the acc banks free before the recip chain
                        nrm_ps = psr.tile([P, T], F32, tag="rot", space="PSUM",
                                          name="nrm_ps")
                        ssbs = {}
                        for h in (h0, h1):
                            base = (h % 2) * DH
                            ssb = spool.tile([1, T], F32, tag="recip", bufs=4, name="ssb")
                            nc.vector.tensor_add(ssb[:], attn_ps[h][DH : DH + 1, :],
                                                 denO[e][base : base + 1, :])
                            nc.vector.tensor_add(attnC[e][base : base + DH, :],
                                                 attn_ps[h][0:DH, :],
                                                 attnC[e][base : base + DH, :])
                            ssbs[h] = ssb
                        for h in (h0, h1):
                            rr = spool.tile([1, T], F32, tag="recip", bufs=4, name="rr")
                            nc.vector.reciprocal_approx_fast(out=rr[:], in_=ssbs[h][:])
                            rrr = spool.tile([1, T], F32R, tag="recip", bufs=4, name="rrr")
                            nc.vector.tensor_copy(rrr[:], rr[:])
                            hsel = hsel_e if h == h0 else hsel_o
                            nc.tensor.matmul(nrm_ps[:], hsel[:], rrr[:],
                                             start=(h == h0), stop=(h == h1))
                        nrm_sb = epool.tile([P, T], F32, tag="lntmp", name="nrm_sb")
                        nc.vector.tensor_copy(nrm_sb[:], nrm_ps[:])
                        for h in (h0, h1):
                            base = (h % 2) * DH
                            nc.vector.tensor_mul(attnC[e][base : base + DH, :],
                                                 attnC[e][base : base + DH, :],
                                                 nrm_sb[base : base + DH, :])

                    # ---- output projection + residual (in place)
                    wo_sb = wpool.tile([P, DT * D], F16, tag="w", name="wo_sb")
                    nc.sync.dma_start(wo_sb[:], woT[l])
                    for m in range(DT):
                        o_ps = psr.tile([P, T], F32, tag="rot", space="PSUM", name="o_ps")
                        for k in range(DT):
                            nc.tensor.matmul(
                                o_ps[:], wo_sb[:, k * D + m * P : k * D + (m + 1) * P],
                                attnC[k][:], start=(k == 0), stop=(k == DT - 1))
                        nc.vector.tensor_add(xT[m][:], o_ps[:], xT[m][:])

                    # ---- FFN: fc1+gelu for all f-tiles, then fc2 m-outer
                    h2T = layer_norm(ln2g, ln2b, l)
                    gT = []
                    for f in range(FT):
                        f1w = fpool.tile([P, DT * P], F16, tag="f1w", name="f1w")
                        nc.sync.dma_start(f1w[:], fc1T[l, f])
                        f1_ps = psr.tile([P, T], F32, tag="rot", space="PSUM", name="f1_ps")
                        for k in range(DT):
                            nc.tensor.matmul(f1_ps[:], f1w[:, k * P : (k + 1) * P],
                                             h2T[k][:], start=(k == 0), stop=(k == DT - 1))
                        g_sb = gpool.tile([P, T], F16, tag="g", name="g_sb")
                        nc.scalar.activation(g_sb[:], f1_ps[:], AF.Gelu_apprx_tanh)
                        gT.append(g_sb)
                    for m in range(DT):
                        x2_ps = psa.tile([P, T], F32, tag="acc", space="PSUM",
                                         name="x2_ps")
                        for f in range(FT):
                            nc.tensor.matmul(x2_ps[:], f2ws[f][:, m * P : (m + 1) * P],
                                             gT[f][:], start=(f == 0), stop=(f == FT - 1))
                        nc.vector.tensor_add(xT[m][:], x2_ps[:], xT[m][:])

            # unembed streaming: issue all chunk loads now so they fill DMA
            # queues during the last layer; consumed at the very end
            uSB = {}
            for ci in range(VCH):
                for k in range(DT):
                    u_sb = upool.tile([P, VCW], F16, tag="us", name="u_sb")
                    nc.sync.dma_start(u_sb[:], uT[k, :, ci * VCW : (ci + 1) * VCW])
                    uSB[(ci, k)] = u_sb

            # ---- final: masked AllReduce of predicted token's x column
            with nc.named_scope("final"):
                cont = dpool.tile([P, DT * B], F32, tag="cont", name="cont")
                csb = spool.tile([P, DT * B], F32, tag="csb", name="csb")
                for k in range(DT):
                    nc.vector.tensor_mul(
                        csb[:, k * B : (k + 1) * B],
                        xT[k][:, pcol : pcol + 1].to_broadcast((P, B)),
                        sel4_sb[:])
                nc.sync.dma_start(cont[:], csb[:])
                ar_out = dpool.tile([P, DT * B], F32, tag="arout",
                                    addr_space="Shared", name="ar_out")
                nc.gpsimd.collective_compute(
                    "AllReduce", OP.add,
                    ins=[cont[:].opt()],
                    outs=[ar_out[:].opt()],
                    replica_groups=[list(range(NC_))],
                )
                xf_raw = spool.tile([P, DT * B], F32, tag="xfraw", name="xf_raw")
                nc.sync.dma_start(xf_raw[:], ar_out[:])
                xf = spool.tile([P, DT * B], F32R, tag="xf", name="xf")
                nc.vector.tensor_copy(xf[:], xf_raw[:])

                lgb = spool.tile([P, 2 * DT], F32, tag="lngb", bufs=3, name="lgb")
                nc.sync.dma_start(lgb[:, 0:DT], lnfg[:])
                nc.sync.dma_start(lgb[:, DT : 2 * DT], lnfb[:])
                fs_ps = psa.tile([1, B], F32, tag="acc", space="PSUM", name="fs_ps")
                fq_ps = psa.tile([1, B], F32, tag="acc", space="PSUM", name="fq_ps")
                xfsq = spool.tile([P, DT * B], F32R, tag="xfsq", name="xfsq")
                nc.vector.tensor_mul(xfsq[:], xf[:], xf[:])
                for k in range(DT):
                    nc.tensor.matmul(fs_ps[:], ones_col[:], xf[:, k * B : (k + 1) * B],
                                     start=(k == 0), stop=(k == DT - 1))
                    nc.tensor.matmul(fq_ps[:], ones_col[:], xfsq[:, k * B : (k + 1) * B],
                                     start=(k == 0), stop=(k == DT - 1))
                fmean = spool.tile([1, B], F32, tag="lnstat", bufs=4, name="fmean")
                nc.vector.tensor_scalar_mul(fmean[:], fs_ps[:], 1.0 / D)
                fm2 = spool.tile([1, B], F32, tag="lnstat", bufs=4, name="fm2")
                nc.vector.tensor_mul(fm2[:], fmean[:], fmean[:])
                fsqd = spool.tile([1, B], F32, tag="lnstat", bufs=4, name="fsqd")
                nc.vector.tensor_scalar_mul(fsqd[:], fq_ps[:], 1.0 / D)
                fvar = spool.tile([1, B], F32, tag="lnstat", bufs=4, name="fvar")
                nc.vector.tensor_sub(fvar[:], fsqd[:], fm2[:])
                fstd = spool.tile([1, B], F32, tag="lnstat", bufs=4, name="fstd")
                nc.scalar.activation(fstd[:], fvar[:], AF.Sqrt, bias=eps1[:])
                # frm = [rstd | -mean*rstd]
                frstd = spool.tile([1, B], F32, tag="lnstat", bufs=4, name="frstd")
                nc.vector.reciprocal(frstd[:], fstd[:])
                frm = spool.tile([1, 2 * B], F32R, tag="lnr", bufs=2, name="frm")
                nc.vector.tensor_copy(frm[0:1, 0:B], frstd[:])
                nc.vector.scalar_tensor_tensor(out=frm[0:1, B : 2 * B],
                                               in0=fmean[:], scalar=-1.0,
                                               in1=frstd[:],
                                               op0=OP.mult, op1=OP.mult)
                fbc_ps = psr.tile([P, 2 * B], F32, tag="rot", space="PSUM",
                                  name="fbc_ps")
                nc.tensor.matmul(fbc_ps[:], ones_row[:], frm[:],
                                 start=True, stop=True)
                xfn = spool.tile([P, DT * B], F16, tag="xfn", name="xfn")
                for k in range(DT):
                    t1 = spool.tile([P, B], F32, tag="lnstat", bufs=4, name="ft1")
                    nc.vector.tensor_mul(t1[:], fbc_ps[:, 0:B],
                                         xf[:, k * B : (k + 1) * B])
                    t2 = spool.tile([P, B], F32, tag="lnstat", bufs=4, name="ft2")
                    nc.vector.tensor_add(t2[:], fbc_ps[:, B : 2 * B], t1[:])
                    nc.scalar.activation(xfn[:, k * B : (k + 1) * B], t2[:], AF.Identity,
                                         scale=lgb[:, k : k + 1],
                                         bias=lgb[:, DT + k : DT + k + 1])

                for ci in range(VCH):
                    lg_ps = psr.tile([B, VCW], F32, tag="rot", space="PSUM", name="lg_ps")
                    for k in range(DT):
                        nc.tensor.matmul(lg_ps[:], xfn[:, k * B : (k + 1) * B],
                                         uSB[(ci, k)][:],
                                         start=(k == 0), stop=(k == DT - 1))
                    och = fpool.tile([B, VCW], F32, tag="och", name="och")
                    nc.vector.tensor_copy(och[:], lg_ps[:])
                    nc.sync.dma_start(out[:, ci * VCW : (ci + 1) * VCW], och[:])

    nc.compile()
    return nc


# ---------------------------------------------------------------- host side
def _positional_encoding(s, d):
    idx = np.arange(d)
    exponent = ((2 * (idx // 2)).astype(np.float32) / float(d)).astype(np.float32)
    pos = np.arange(s, dtype=np.float32)[:, None]
    angle = pos / np.power(np.float32(10000.0), exponent[None, :], dtype=np.float32)
    return np.where((idx % 2 == 0)[None, :], np.sin(angle), np.cos(angle)).astype(np.float32)


def _build_masks():
    """trimask[r, c] = 1 if key r <= query c (within-block causal)."""
    r = np.arange(P)
    return (r[:, None] <= r[None, :]).astype(np.float16)


def prepare_inputs(tokens, predict_idx, embedding, ln1_g, ln1_b, wq, wk, wv, wo,
                   ln2_g, ln2_b, fc1, fc2, lnf_g, lnf_b, unembed, n_layers=NL):
    f = lambda a: np.ascontiguousarray(np.asarray(a), dtype=np.float32)
    tokens = np.asarray(tokens)
    emb = f(embedding)
    pos = _positional_encoding(S, D)

    def wlayout(a):  # [L, out, in] -> [L, P, DT*D] with [l, p, k*D + dout]
        aT = a.transpose(0, 2, 1)
        return np.ascontiguousarray(
            aT.reshape(n_layers, DT, P, D).transpose(0, 2, 1, 3)
            .reshape(n_layers, P, DT * D)).astype(np.float16)

    wqT = wlayout(f(wq)[:n_layers].reshape(-1, NH * DH, D))
    wkT = wlayout(f(wk)[:n_layers].reshape(-1, NH * DH, D))
    wvT = wlayout(f(wv)[:n_layers].reshape(-1, NH * DH, D))
    woT = wlayout(f(wo)[:n_layers])
    fc1T = np.ascontiguousarray(
        f(fc1)[:n_layers].transpose(0, 2, 1)
        .reshape(n_layers, DT, P, FT, P).transpose(0, 3, 2, 1, 4)
        .reshape(n_layers, FT, P, DT * P)).astype(np.float16)
    fc2T = np.ascontiguousarray(
        f(fc2)[:n_layers].transpose(0, 2, 1)
        .reshape(n_layers, FT, P, D)).astype(np.float16)
    uTf = np.ascontiguousarray(f(unembed).T.reshape(DT, P, V)).astype(np.float16)

    def lnshape(a):
        return np.ascontiguousarray(
            f(a)[:n_layers].reshape(n_layers, DT, P).transpose(0, 2, 1))

    lns = {
        "ln1g": lnshape(ln1_g), "ln1b": lnshape(ln1_b),
        "ln2g": lnshape(ln2_g), "ln2b": lnshape(ln2_b),
        "lnfg": np.ascontiguousarray(f(lnf_g).reshape(DT, P).T),
        "lnfb": np.ascontiguousarray(f(lnf_b).reshape(DT, P).T),
    }
    masks = _build_masks()

    pidx = int(predict_idx)
    in_maps = []
    for c in range(NC_):
        b, h = c // 2, c % 2
        toks = np.asarray(tokens[b, h * T : (h + 1) * T]).astype(np.int64)
        x0 = emb.T[toks] + pos[h * T : (h + 1) * T]
        x0T = np.ascontiguousarray(x0.T.reshape(DT, P, T)).astype(np.float32)
        sel4 = np.zeros((P, B), np.float32)
        if pidx // T == h:
            sel4[:, b] = 1.0
        m = {
            "x0T": x0T, "wqT": wqT, "wkT": wkT, "wvT": wvT, "woT": woT,
            "fc1T": fc1T, "fc2T": fc2T,
            "uT": uTf[:, :, c * VS : (c + 1) * VS].copy(),
            "masks": masks,
            "remw": np.full((P, 1), 1.0 if h == 1 else 0.0, np.float32),
            "sel4": sel4,
            "pairsel": np.array([[(c % 2) ^ 1]], np.int32),
            **lns,
        }
        in_maps.append(m)
    return in_maps


_CACHED = {}


def kernel(**inputs):
    from concourse.bass_utils import run_bass_kernel_spmd
    pidx = int(np.asarray(inputs["predict_idx"]))
    key = ("nc", pidx % T)
    if key not in _CACHED:
        _CACHED[key] = build_nc(pcol=pidx % T)
    nc = _CACHED[key]
    in_maps = prepare_inputs(**inputs)
    res = run_bass_kernel_spmd(nc, in_maps, core_ids=list(range(NC_)), trace=False)
    return np.concatenate([res.results[c]["out"] for c in range(NC_)], axis=1)


# revision 28
# speedup vs baseline: 1.0234x; 1.0234x over previous
"""MicroGPT forward pass on 8 Trainium2 NeuronCores (Bass/Tile).

Sharding: token-sharded — core c = 2*b + h owns batch b, sequence half h
(512 contiguous tokens). Activations are feature-major in SBUF
(x^T: [768 rows -> 6 tiles of 128, 512 token cols]); all matmuls fp32r/f16.
Attention: S^T = K^T-slice (stationary) x Q^T (moving); softmax without max
subtraction (scores bounded); denominators via a ones column appended to V.
K/V slots 0-3 are the core's own 4 blocks (block-causal masks, identical on
every core); slots 4-7 are the pair core's blocks, weighted by a per-core
0/1 scalar (1 when the pair holds earlier positions). Per layer a PAIRWISE
(2-rank) AllGather shares K^T/V; each core reads its pair's shard with a
register-indexed dynamic DMA. All per-token statistic broadcasts (LN
rstd/mean, softmax reciprocal) are done with K=1 matmuls on the PE instead
of DMA round-trips. Gelu is a single fused ACT op; the FFN runs fc1 for all
24 f-tiles first, then fc2 m-outer with all fc2 weights SBUF-resident so
the PE streams uninterrupted. Final token: masked AllReduce, then final LN +
vocab-sharded unembed (4000 vocab rows per core, preloaded to SBUF).
"""
import sys, math

sys.path.insert(0, "/opt/trn_rl_repo")
import numpy as np

import concourse.bass as bass
import concourse.bacc as bacc
import concourse.mybir as mybir
import concourse.tile as tile

D, NH, DH, FF, NL, V = 768, 12, 64, 3072, 4, 32000
B, S = 4, 1024
EPS = 1e-5
NC_ = 8
P = 128
T = 512            # tokens per core
DT = D // P        # 6 d-model tiles
FT = FF // P       # 24 ff tiles
KB = 8             # key slots (0-3 own, 4-7 pair)
VS = V // NC_      # 4000 vocab rows per core
VCH = 8            # vocab chunks of 500
VCW = VS // VCH    # 500
F32 = mybir.dt.float32
F32R = mybir.dt.float32r
F16 = mybir.dt.float16
BF16 = mybir.dt.bfloat16
I32 = mybir.dt.int32
AF = mybir.ActivationFunctionType
OP = mybir.AluOpType
SCALE = 1.0 / math.sqrt(DH)
VW = NH * (DH + 1)           # 780 — V tile width incl. ones cols
CONTRIB_W = DT * T + 4 * VW  # AllGather contribution width


# ---------------------------------------------------------------- bass program
def build_nc(n_layers=NL, pcol=511, dbg=False):
    nc = bacc.Bacc(None, target_bir_lowering=False, debug=False, num_devices=NC_)

    x0T = nc.dram_tensor("x0T", [DT, P, T], F32R, kind="ExternalInput")
    wqT = nc.dram_tensor("wqT", [n_layers, P, DT * D], F16, kind="ExternalInput")
    wkT = nc.dram_tensor("wkT", [n_layers, P, DT * D], F16, kind="ExternalInput")
    wvT = nc.dram_tensor("wvT", [n_layers, P, DT * D], F16, kind="ExternalInput")
    woT = nc.dram_tensor("woT", [n_layers, P, DT * D], F16, kind="ExternalInput")
    fc1T = nc.dram_tensor("fc1T", [n_layers, FT, P, DT * P], F16, kind="ExternalInput")
    fc2T = nc.dram_tensor("fc2T", [n_layers, FT, P, D], F16, kind="ExternalInput")
    ln1g = nc.dram_tensor("ln1g", [n_layers, P, DT], F32, kind="ExternalInput")
    ln1b = nc.dram_tensor("ln1b", [n_layers, P, DT], F32, kind="ExternalInput")
    ln2g = nc.dram_tensor("ln2g", [n_layers, P, DT], F32, kind="ExternalInput")
    ln2b = nc.dram_tensor("ln2b", [n_layers, P, DT], F32, kind="ExternalInput")
    lnfg = nc.dram_tensor("lnfg", [P, DT], F32, kind="ExternalInput")
    lnfb = nc.dram_tensor("lnfb", [P, DT], F32, kind="ExternalInput")
    uT = nc.dram_tensor("uT", [DT, P, VS], F16, kind="ExternalInput")
    masks = nc.dram_tensor("masks", [P, P], F16, kind="ExternalInput")
    remw = nc.dram_tensor("remw", [P, 1], F32, kind="ExternalInput")
    sel4 = nc.dram_tensor("sel4", [P, B], F32R, kind="ExternalInput")
    pairsel = nc.dram_tensor("pairsel", [1, 1], I32, kind="ExternalInput")

    out = nc.dram_tensor("out", [B, VS], F32, kind="ExternalOutput")

    from contextlib import ExitStack
    with tile.TileContext(nc) as tc:
        with ExitStack() as _stk:
            _p = lambda *a, **kw: _stk.enter_context(tc.tile_pool(*a, **kw))
            cpool = _p(name="const", bufs=1)
            ppool = _p(name="persist", bufs=1)
            xpool = _p(name="xp", bufs=6)
            hpool = _p(name="hp", bufs=6)
            qpool = _p(name="qp", bufs=6)
            apool = _p(name="ac", bufs=6)
            wpool = _p(name="wp", bufs=3)
            fpool = _p(name="fp", bufs=3)
            f2pool = _p(name="f2p", bufs=FT)
            gpool = _p(name="gp", bufs=FT)
            upool = _p(name="up", bufs=4)
            epool = _p(name="ep", bufs=4)
            spool = _p(name="sp", bufs=5)
            psr = _p(name="psr", bufs=2, space="PSUM")
            psa = _p(name="psa", bufs=2, space="PSUM")
            psb = _p(name="psb", bufs=2, space="PSUM")
            dpool = _p(name="dram", bufs=2, space="DRAM")
            # ---- constants (memset cannot write f32r; stage via f32 + copy)
            ones_f32 = cpool.tile([P, 1], F32)
            nc.vector.memset(ones_f32[:], 1.0)
            trimask = cpool.tile([P, P], F16)
            nc.sync.dma_start(trimask[:], masks[:])
            ones_col = cpool.tile([P, 1], F32R)
            nc.vector.tensor_copy(ones_col[:], ones_f32[:])
            onesr_f32 = cpool.tile([1, P], F32)
            nc.vector.memset(onesr_f32[:], 1.0)
            ones_row = cpool.tile([1, P], F32R)
            nc.vector.tensor_copy(ones_row[:], onesr_f32[:])
            eps1 = cpool.tile([1, 1], F32)
            nc.vector.memset(eps1[:], EPS)
            hse_f = cpool.tile([1, P], F32)
            nc.vector.memset(hse_f[:], 0.0)
            nc.vector.memset(hse_f[0:1, 0:DH], 1.0)
            hsel_e = cpool.tile([1, P], F32R)
            nc.vector.tensor_copy(hsel_e[:], hse_f[:])
            hso_f = cpool.tile([1, P], F32)
            nc.vector.memset(hso_f[:], 0.0)
            nc.vector.memset(hso_f[0:1, DH:P], 1.0)
            hsel_o = cpool.tile([1, P], F32R)
            nc.vector.tensor_copy(hsel_o[:], hso_f[:])
            sel4_sb = cpool.tile([P, B], F32R)
            nc.sync.dma_start(sel4_sb[:], sel4[:])
            remw_sb = cpool.tile([P, 1], F32)
            nc.sync.dma_start(remw_sb[:], remw[:])

            # persistent K^T / V buffers (slots 0-3 own, 4-7 pair)
            KT = [ppool.tile([P, KB * P], F16, tag=f"kt{e}", name=f"KT{e}")
                  for e in range(DT)]
            VT = [ppool.tile([P, VW], F16, tag=f"vt{j}", name=f"VT{j}")
                  for j in range(KB)]
            for j in range(4):
                for h in range(NH):
                    nc.vector.tensor_copy(
                        VT[j][:, h * (DH + 1) + DH : h * (DH + 1) + DH + 1],
                        ones_f32[:])

            # pair rank register for dynamic reads of the AllGather output
            with tc.tile_critical():
                with nc.sync.register("pairreg") as preg:
                    nc.sync.reg_load(preg, pairsel[0:1, 0:1])
                    pv = nc.sync.snap(preg, min_val=0, max_val=1)

            # ---- residual stream (updated in place by residual adds)
            xT = []
            for k in range(DT):
                t_ = xpool.tile([P, T], F32R, tag="xT", name=f"xT{k}")
                nc.sync.dma_start(t_[:], x0T[k])
                xT.append(t_)

            def layer_norm(g_dram, b_dram, l):
                """mean/var via PE stats matmuls; rstd & -mean*rstd broadcast
                to all partitions with a K=1 matmul (no DMA round-trip)."""
                gb = spool.tile([P, 2 * DT], F32, tag="lngb", bufs=3, name="gb")
                nc.sync.dma_start(gb[:, 0:DT], g_dram[l])
                nc.sync.dma_start(gb[:, DT : 2 * DT], b_dram[l])
                sum_ps = psa.tile([1, T], F32, tag="acc", space="PSUM", name="sum_ps")
                sq_ps = psa.tile([1, T], F32, tag="acc", space="PSUM", name="sq_ps")
                sum_ps, sq_ps = sum_ps[:], sq_ps[:]
                for k in range(DT):
                    xsq = epool.tile([P, T], F32R, tag="lntmp", name="xsq")
                    nc.vector.tensor_mul(xsq[:], xT[k][:], xT[k][:])
                    nc.tensor.matmul(sum_ps, ones_col[:], xT[k][:],
                                     start=(k == 0), stop=(k == DT - 1))
                    nc.tensor.matmul(sq_ps, ones_col[:], xsq[:],
                                     start=(k == 0), stop=(k == DT - 1))
                sums_sb = spool.tile([1, T], F32, tag="lnstat", bufs=4, name="sums_sb")
                nc.vector.tensor_copy(sums_sb[:], sum_ps)
                m2s = spool.tile([1, T], F32, tag="lnstat", bufs=4, name="m2s")
                nc.vector.scalar_tensor_tensor(out=m2s[:], in0=sums_sb[:],
                                               scalar=1.0 / (D * D), in1=sums_sb[:],
                                               op0=OP.mult, op1=OP.mult)
                var = spool.tile([1, T], F32, tag="lnstat", bufs=4, name="var")
                nc.vector.scalar_tensor_tensor(out=var[:], in0=sq_ps,
                                               scalar=1.0 / D, in1=m2s[:],
                                               op0=OP.mult, op1=OP.subtract)
                std = spool.tile([1, T], F32, tag="lnstat", bufs=4, name="std")
                nc.scalar.activation(std[:], var[:], AF.Sqrt, bias=eps1[:])
                # rm = [rstd | -mean*rstd] in one f32r row
                rstd = spool.tile([1, T], F32, tag="lnstat", bufs=4, name="rstd")
                nc.vector.reciprocal_approx_fast(out=rstd[:], in_=std[:])
                rm = spool.tile([1, 2 * T], F32R, tag="lnr", bufs=2, name="rm")
                nc.vector.tensor_copy(rm[0:1, 0:T], rstd[:])
                nc.vector.scalar_tensor_tensor(out=rm[0:1, T : 2 * T],
                                               in0=sums_sb[:], scalar=-1.0 / D,
                                               in1=rstd[:],
                                               op0=OP.mult, op1=OP.mult)
                bc_ps = psb.tile([P, 2 * T], F32, tag="bc", space="PSUM",
                                 name="bc_ps")
                nc.tensor.matmul(bc_ps[:, 0:T], ones_row[:],
                                 rm[0:1, 0:T], start=True, stop=True)
                nc.tensor.matmul(bc_ps[:, T : 2 * T], ones_row[:],
                                 rm[0:1, T : 2 * T], start=True, stop=True)
                hT = []
                for k in range(DT):
                    t1 = epool.tile([P, T], F32, tag="lntmp", name="lnt1")
                    nc.vector.tensor_mul(t1[:], bc_ps[:, 0:T], xT[k][:])
                    t2 = epool.tile([P, T], F32, tag="lntmp", name="lnt2")
                    nc.vector.tensor_add(t2[:], bc_ps[:, T : 2 * T], t1[:])
                    h_ = hpool.tile([P, T], F16, tag="hT", name="hT_t")
                    nc.scalar.activation(h_[:], t2[:], AF.Identity,
                                         scale=gb[:, k : k + 1],
                                         bias=gb[:, DT + k : DT + k + 1])
                    hT.append(h_)
                return hT

            for l in range(n_layers):
                with nc.named_scope(f"L{l}"):
                    hT = layer_norm(ln1g, ln1b, l)

                    # ---- K^T, V first (feeds AllGather early), then Q^T
                    wk_sb = wpool.tile([P, DT * D], F16, tag="w", name="wk_sb")
                    nc.sync.dma_start(wk_sb[:], wkT[l])
                    for m in range(DT):
                        k_ps = psr.tile([P, T], F32, tag="rot", space="PSUM", name="k_ps")
                        for k in range(DT):
                            nc.tensor.matmul(
                                k_ps[:], wk_sb[:, k * D + m * P : k * D + (m + 1) * P],
                                hT[k][:], start=(k == 0), stop=(k == DT - 1))
                        nc.vector.tensor_copy(KT[m][:, 0:T], k_ps[:])

                    wv_sb = wpool.tile([P, DT * D], F16, tag="w", name="wv_sb")
                    nc.sync.dma_start(wv_sb[:], wvT[l])
                    for m in range(4):
                        for c in range(2):
                            v_ps = psr.tile([P, 6 * DH], F32, tag="rot", space="PSUM",
                                            name="v_ps")
                            for k in range(DT):
                                nc.tensor.matmul(
                                    v_ps[:], hT[k][:, m * P : (m + 1) * P],
                                    wv_sb[:, k * D + c * 6 * DH : k * D + (c + 1) * 6 * DH],
                                    start=(k == 0), stop=(k == DT - 1))
                            dst = VT[m][:, c * 6 * (DH + 1) : (c + 1) * 6 * (DH + 1)] \
                                .rearrange("p (h e) -> p h e", h=6, e=DH + 1)[:, :, 0:DH]
                            src = v_ps[:].rearrange("p (h e) -> p h e", h=6, e=DH)
                            nc.vector.tensor_copy(dst, src)

                    # ---- share K^T/V with the pair core (2-rank AllGather)
                    contrib = dpool.tile([P, CONTRIB_W], F16, tag="contrib", name="contrib")
                    for e in range(DT):
                        nc.sync.dma_start(contrib[:, e * T : (e + 1) * T],
                                          KT[e][:, 0:T])
                    for m in range(4):
                        nc.sync.dma_start(
                            contrib[:, DT * T + m * VW : DT * T + (m + 1) * VW],
                            VT[m][:])
                    gout = dpool.tile([2, P, CONTRIB_W], F16, tag="gout",
                                      name="gout")
                    nc.gpsimd.collective_compute(
                        "AllGather", OP.bypass,
                        ins=[contrib[:].opt()],
                        outs=[gout[:].opt()],
                        replica_groups=[[2 * i, 2 * i + 1] for i in range(4)],
                    )
                    rsrc = gout[bass.ds(pv, 1)]
                    for e in range(DT):
                        nc.sync.dma_start(KT[e][:, T : 2 * T],
                                          rsrc[0, :, e * T : (e + 1) * T])
                    for m in range(4):
                        nc.sync.dma_start(
                            VT[4 + m][:],
                            rsrc[0, :, DT * T + m * VW : DT * T + (m + 1) * VW])
                        nc.vector.tensor_scalar_mul(VT[4 + m][:], VT[4 + m][:],
                                                    remw_sb[:, 0:1])

                    # prefetch fc2 weights during attention
                    f2ws = []
                    for f in range(FT):
                        f2w = f2pool.tile([P, D], F16, tag="f2w", name="f2w")
                        nc.sync.dma_start(f2w[:], fc2T[l, f])
                        f2ws.append(f2w)

                    wq_sb = wpool.tile([P, DT * D], F16, tag="w", name="wq_sb")
                    nc.sync.dma_start(wq_sb[:], wqT[l])
                    QT = []
                    for m in range(DT):
                        q_ps = psr.tile([P, T], F32, tag="rot", space="PSUM", name="q_ps")
                        for k in range(DT):
                            nc.tensor.matmul(
                                q_ps[:], wq_sb[:, k * D + m * P : k * D + (m + 1) * P],
                                hT[k][:], start=(k == 0), stop=(k == DT - 1))
                        qt = qpool.tile([P, T], F16, tag="qt", name="qt")
                        nc.vector.tensor_copy(qt[:], q_ps[:])
                        QT.append(qt)

                    # ---- attention, head pairs (2 heads share one attnC tile)
                    # own slots j<4: only queries >= slot start (suffix);
                    # remote slots: full width, merged j={4,5} / {6,7} into one
                    # [P, 2T] score tile -> single exp; V zeroed on h=0 cores.
                    attnC = [apool.tile([P, T], F16, tag="attnC", name=f"attnC{e}")
                             for e in range(DT)]
                    # phase 1: OWN slots for every pair (no AllGather dependency
                    # -> runs during the collective); partial numerators land in
                    # attnC, denominators at partitions 0/64 of a small tile.
                    denO = []
                    for e in range(DT):
                        h0, h1 = 2 * e, 2 * e + 1
                        attn_ps = {
                            h0: psa.tile([DH + 1, T], F32, tag="acc",
                                         space="PSUM", name=f"attnps{h0}"),
                            h1: psa.tile([DH + 1, T], F32, tag="acc",
                                         space="PSUM", name=f"attnps{h1}"),
                        }
                        for j in range(4):
                            c0 = j * P
                            N = T - c0
                            sps = {}
                            for h in (h0, h1):
                                base = (h % 2) * DH
                                s_ps = psr.tile([P, T], F32, tag="rot",
                                                space="PSUM", name="s_ps")
                                nc.tensor.matmul(
                                    s_ps[:, 0:N],
                                    KT[e][base : base + DH, j * P : (j + 1) * P],
                                    QT[e][base : base + DH, c0:T],
                                    start=True, stop=True)
                                sps[h] = s_ps
                            es = {}
                            for h in (h0, h1):
                                e_sb = epool.tile([P, T], F16, tag="e", name="e_sb")
                                nc.scalar.activation(e_sb[:, 0:N], sps[h][:, 0:N],
                                                     AF.Exp, scale=SCALE)
                                nc.vector.tensor_mul(e_sb[:, 0:P], e_sb[:, 0:P],
                                                     trimask[:])
                                es[h] = e_sb
                            for h in (h0, h1):
                                nc.tensor.matmul(
                                    attn_ps[h][:, c0:T],
                                    VT[j][:, h * (DH + 1) : (h + 1) * (DH + 1)],
                                    es[h][:, 0:N],
                                    start=(j == 0), stop=(j == 3))
                        dp = spool.tile([DH + 1, T], F16, tag="deno", bufs=6,
                                        name="dp")
                        for h in (h0, h1):
                            base = (h % 2) * DH
                            nc.vector.tensor_copy(attnC[e][base : base + DH, :],
                                                  attn_ps[h][0:DH, :])
                            nc.vector.tensor_copy(dp[base : base + 1, :],
                                                  attn_ps[h][DH : DH + 1, :])
                        denO.append(dp)
                    # phase 2: REMOTE slots (pair data) + combine + normalize
                    for e in range(DT):
                        h0, h1 = 2 * e, 2 * e + 1
                        attn_ps = {
                            h0: psa.tile([DH + 1, T], F32, tag="acc",
                                         space="PSUM", name=f"attnpr{h0}"),
                            h1: psa.tile([DH + 1, T], F32, tag="acc",
                                         space="PSUM", name=f"attnpr{h1}"),
                        }
                        for jp in (4, 6):
                            sws = {}
                            for h in (h0, h1):
                                sws[h] = psb.tile([P, 2 * T], F32, tag="bc",
                                                  space="PSUM", name="sw")
                            for jo in (0, 1):
                                for h in (h0, h1):
                                    base = (h % 2) * DH
                                    nc.tensor.matmul(
                                        sws[h][:, jo * T : (jo + 1) * T],
                                        KT[e][base : base + DH,
                                              (jp + jo) * P : (jp + jo + 1) * P],
                                        QT[e][base : base + DH, :],
                                        start=True, stop=True)
                            ews = {}
                            for h in (h0, h1):
                                ew = epool.tile([P, 2 * T], F16, tag="e", name="ew")
                                nc.scalar.activation(ew[:], sws[h][:], AF.Exp,
                                                     scale=SCALE)
                                ews[h] = ew
                            for h in (h0, h1):
                                nc.tensor.matmul(
                                    attn_ps[h][:],
                                    VT[jp][:, h * (DH + 1) : (h + 1) * (DH + 1)],
                                    ews[h][:, 0:T],
                                    start=(jp == 4), stop=False)
                                nc.tensor.matmul(
                                    attn_ps[h][:],
                                    VT[jp + 1][:, h * (DH + 1) : (h + 1) * (DH + 1)],
                                    ews[h][:, T : 2 * T],
                                    start=False, stop=(jp == 6))
                        nrm_ps = psr.tile([P, T], F32, tag="rot", space="PSUM",
                                          name="nrm_ps")
                        for h in (h0, h1):
                            base = (h % 2) * DH
                            ssb = spool.tile([1, T], F32, tag="recip", bufs=4, name="ssb")
                            nc.vector.tensor_add(ssb[:], attn_ps[h][DH : DH + 1, :],
                                                 denO[e][base : base + 1, :])
                            rr = spool.tile([1, T], F32, tag="recip", bufs=4, name="rr")
                            nc.vector.reciprocal_approx_fast(out=rr[:], in_=ssb[:])
                            rrr = spool.tile([1, T], F32R, tag="recip", bufs=4, name="rrr")
                            nc.vector.tensor_copy(rrr[:], rr[:])
                            hsel = hsel_e if h == h0 else hsel_o
                            nc.tensor.matmul(nrm_ps[:], hsel[:], rrr[:],
                                             start=(h == h0), stop=(h == h1))
                        nrm_sb = epool.tile([P, T], F32, tag="lntmp", name="nrm_sb")
                        nc.vector.tensor_copy(nrm_sb[:], nrm_ps[:])
                        for h in (h0, h1):
                            base = (h % 2) * DH
                            nc.vector.tensor_add(attnC[e][base : base + DH, :],
                                                 attn_ps[h][0:DH, :],
                                                 attnC[e][base : base + DH, :])
                            nc.vector.tensor_mul(attnC[e][base : base + DH, :],
                                                 attnC[e][base : base + DH, :],
                                                 nrm_sb[base : base + DH, :])

                    # ---- output projection + residual (in place)
                    wo_sb = wpool.tile([P, DT * D], F16, tag="w", name="wo_sb")
                    nc.sync.dma_start(wo_sb[:], woT[l])
                    for m in range(DT):
                        o_ps = psr.tile([P, T], F32, tag="rot", space="PSUM", name="o_ps")
                        for k in range(DT):
                            nc.tensor.matmul(
                                o_ps[:], wo_sb[:, k * D + m * P : k * D + (m + 1) * P],
                                attnC[k][:], start=(k == 0), stop=(k == DT - 1))
                        nc.vector.tensor_add(xT[m][:], o_ps[:], xT[m][:])

                    # ---- FFN: fc1+gelu for all f-tiles, then fc2 m-outer
                    h2T = layer_norm(ln2g, ln2b, l)
                    gT = []
                    for f in range(FT):
                        f1w = fpool.tile([P, DT * P], F16, tag="f1w", name="f1w")
                        nc.sync.dma_start(f1w[:], fc1T[l, f])
                        f1_ps = psr.tile([P, T], F32, tag="rot", space="PSUM", name="f1_ps")
                        for k in range(DT):
                            nc.tensor.matmul(f1_ps[:], f1w[:, k * P : (k + 1) * P],
                                             h2T[k][:], start=(k == 0), stop=(k == DT - 1))
                        g_sb = gpool.tile([P, T], F16, tag="g", name="g_sb")
                        nc.scalar.activation(g_sb[:], f1_ps[:], AF.Gelu_apprx_tanh)
                        gT.append(g_sb)
                    for m in range(DT):
                        x2_ps = psa.tile([P, T], F32, tag="acc", space="PSUM",
                                         name="x2_ps")
                        for f in range(FT):
                            nc.tensor.matmul(x2_ps[:], f2ws[f][:, m * P : (m + 1) * P],
                                             gT[f][:], start=(f == 0), stop=(f == FT - 1))
                        nc.vector.tensor_add(xT[m][:], x2_ps[:], xT[m][:])

            # unembed streaming: issue all chunk loads now so they fill DMA
            # queues during the last layer; consumed at the very end
            uSB = {}
            for ci in range(VCH):
                for k in range(DT):
                    u_sb = upool.tile([P, VCW], F16, tag="us", name="u_sb")
                    nc.sync.dma_start(u_sb[:], uT[k, :, ci * VCW : (ci + 1) * VCW])
                    uSB[(ci, k)] = u_sb

            # ---- final: masked AllReduce of predicted token's x column
            with nc.named_scope("final"):
                cont = dpool.tile([P, DT * B], F32, tag="cont", name="cont")
                csb = spool.tile([P, DT * B], F32, tag="csb", name="csb")
                for k in range(DT):
                    nc.vector.tensor_mul(
                        csb[:, k * B : (k + 1) * B],
                        xT[k][:, pcol : pcol + 1].to_broadcast((P, B)),
                        sel4_sb[:])
                nc.sync.dma_start(cont[:], csb[:])
                ar_out = dpool.tile([P, DT * B], F32, tag="arout",
                                    addr_space="Shared", name="ar_out")
                nc.gpsimd.collective_compute(
                    "AllReduce", OP.add,
                    ins=[cont[:].opt()],
                    outs=[ar_out[:].opt()],
                    replica_groups=[list(range(NC_))],
                )
                xf_raw = spool.tile([P, DT * B], F32, tag="xfraw", name="xf_raw")
                nc.sync.dma_start(xf_raw[:], ar_out[:])
                xf = spool.tile([P, DT * B], F32R, tag="xf", name="xf")
                nc.vector.tensor_copy(xf[:], xf_raw[:])

                lgb = spool.tile([P, 2 * DT], F32, tag="lngb", bufs=3, name="lgb")
                nc.sync.dma_start(lgb[:, 0:DT], lnfg[:])
                nc.sync.dma_start(lgb[:, DT : 2 * DT], lnfb[:])
                fs_ps = psa.tile([1, B], F32, tag="acc", space="PSUM", name="fs_ps")
                fq_ps = psa.tile([1, B], F32, tag="acc", space="PSUM", name="fq_ps")
                xfsq = spool.tile([P, DT * B], F32R, tag="xfsq", name="xfsq")
                nc.vector.tensor_mul(xfsq[:], xf[:], xf[:])
                for k in range(DT):
                    nc.tensor.matmul(fs_ps[:], ones_col[:], xf[:, k * B : (k + 1) * B],
                                     start=(k == 0), stop=(k == DT - 1))
                    nc.tensor.matmul(fq_ps[:], ones_col[:], xfsq[:, k * B : (k + 1) * B],
                                     start=(k == 0), stop=(k == DT - 1))
                fmean = spool.tile([1, B], F32, tag="lnstat", bufs=4, name="fmean")
                nc.vector.tensor_scalar_mul(fmean[:], fs_ps[:], 1.0 / D)
                fm2 = spool.tile([1, B], F32, tag="lnstat", bufs=4, name="fm2")
                nc.vector.tensor_mul(fm2[:], fmean[:], fmean[:])
                fsqd = spool.tile([1, B], F32, tag="lnstat", bufs=4, name="fsqd")
                nc.vector.tensor_scalar_mul(fsqd[:], fq_ps[:], 1.0 / D)
                fvar = spool.tile([1, B], F32, tag="lnstat", bufs=4, name="fvar")
                nc.vector.tensor_sub(fvar[:], fsqd[:], fm2[:])
                fstd = spool.tile([1, B], F32, tag="lnstat", bufs=4, name="fstd")
                nc.scalar.activation(fstd[:], fvar[:], AF.Sqrt, bias=eps1[:])
                # frm = [rstd | -mean*rstd]
                frstd = spool.tile([1, B], F32, tag="lnstat", bufs=4, name="frstd")
                nc.vector.reciprocal(frstd[:], fstd[:])
                frm = spool.tile([1, 2 * B], F32R, tag="lnr", bufs=2, name="frm")
                nc.vector.tensor_copy(frm[0:1, 0:B], frstd[:])
                nc.vector.scalar_tensor_tensor(out=frm[0:1, B : 2 * B],
                                               in0=fmean[:], scalar=-1.0,
                                               in1=frstd[:],
                                               op0=OP.mult, op1=OP.mult)
                fbc_ps = psr.tile([P, 2 * B], F32, tag="rot", space="PSUM",
                                  name="fbc_ps")
                nc.tensor.matmul(fbc_ps[:], ones_row[:], frm[:],
                                 start=True, stop=True)
                xfn = spool.tile([P, DT * B], F16, tag="xfn", name="xfn")
                for k in range(DT):
                    t1 = spool.tile([P, B], F32, tag="lnstat", bufs=4, name="ft1")
                    nc.vector.tensor_mul(t1[:], fbc_ps[:, 0:B],
                                         xf[:, k * B : (k + 1) * B])
                    t2 = spool.tile([P, B], F32, tag="lnstat", bufs=4, name="ft2")
                    nc.vector.tensor_add(t2[:], fbc_ps[:, B : 2 * B], t1[:])
                    nc.scalar.activation(xfn[:, k * B : (k + 1) * B], t2[:], AF.Identity,
                                         scale=lgb[:, k : k + 1],
                                         bias=lgb[:, DT + k : DT + k + 1])

                for ci in range(VCH):
                    lg_ps = psr.tile([B, VCW], F32, tag="rot", space="PSUM", name="lg_ps")
                    for k in range(DT):
                        nc.tensor.matmul(lg_ps[:], xfn[:, k * B : (k + 1) * B],
                                         uSB[(ci, k)][:],
                                         start=(k == 0), stop=(k == DT - 1))
                    och = fpool.tile([B, VCW], F32, tag="och", name="och")
                    nc.vector.tensor_copy(och[:], lg_ps[:])
                    nc.sync.dma_start(out[:, ci * VCW : (ci + 1) * VCW], och[:])

    nc.compile()
    return nc


# ---------------------------------------------------------------- host side
def _positional_encoding(s, d):
    idx = np.arange(d)
    exponent = ((2 * (idx // 2)).astype(np.float32) / float(d)).astype(np.float32)
    pos = np.arange(s, dtype=np.float32)[:, None]
    angle = pos / np.power(np.float32(10000.0), exponent[None, :], dtype=np.float32)
    return np.where((idx % 2 == 0)[None, :], np.sin(angle), np.cos(angle)).astype(np.float32)


def _build_masks():
    """trimask[r, c] = 1 if key r <= query c (within-block causal)."""
    r = np.arange(P)
    return (r[:, None] <= r[None, :]).astype(np.float16)


def prepare_inputs(tokens, predict_idx, embedding, ln1_g, ln1_b, wq, wk, wv, wo,
                   ln2_g, ln2_b, fc1, fc2, lnf_g, lnf_b, unembed, n_layers=NL):
    f = lambda a: np.ascontiguousarray(np.asarray(a), dtype=np.float32)
    tokens = np.asarray(tokens)
    emb = f(embedding)
    pos = _positional_encoding(S, D)

    def wlayout(a):  # [L, out, in] -> [L, P, DT*D] with [l, p, k*D + dout]
        aT = a.transpose(0, 2, 1)
        return np.ascontiguousarray(
            aT.reshape(n_layers, DT, P, D).transpose(0, 2, 1, 3)
            .reshape(n_layers, P, DT * D)).astype(np.float16)

    wqT = wlayout(f(wq)[:n_layers].reshape(-1, NH * DH, D))
    wkT = wlayout(f(wk)[:n_layers].reshape(-1, NH * DH, D))
    wvT = wlayout(f(wv)[:n_layers].reshape(-1, NH * DH, D))
    woT = wlayout(f(wo)[:n_layers])
    fc1T = np.ascontiguousarray(
        f(fc1)[:n_layers].transpose(0, 2, 1)
        .reshape(n_layers, DT, P, FT, P).transpose(0, 3, 2, 1, 4)
        .reshape(n_layers, FT, P, DT * P)).astype(np.float16)
    fc2T = np.ascontiguousarray(
        f(fc2)[:n_layers].transpose(0, 2, 1)
        .reshape(n_layers, FT, P, D)).astype(np.float16)
    uTf = np.ascontiguousarray(f(unembed).T.reshape(DT, P, V)).astype(np.float16)

    def lnshape(a):
        return np.ascontiguousarray(
            f(a)[:n_layers].reshape(n_layers, DT, P).transpose(0, 2, 1))

    lns = {
        "ln1g": lnshape(ln1_g), "ln1b": lnshape(ln1_b),
        "ln2g": lnshape(ln2_g), "ln2b": lnshape(ln2_b),
        "lnfg": np.ascontiguousarray(f(lnf_g).reshape(DT, P).T),
        "lnfb": np.ascontiguousarray(f(lnf_b).reshape(DT, P).T),
    }
    masks = _build_masks()

    pidx = int(predict_idx)
    in_maps = []
    for c in range(NC_):
        b, h = c // 2, c % 2
        toks = np.asarray(tokens[b, h * T : (h + 1) * T]).astype(np.int64)
        x0 = emb.T[toks] + pos[h * T : (h + 1) * T]
        x0T = np.ascontiguousarray(x0.T.reshape(DT, P, T)).astype(np.float32)
        sel4 = np.zeros((P, B), np.float32)
        if pidx // T == h:
            sel4[:, b] = 1.0
        m = {
            "x0T": x0T, "wqT": wqT, "wkT": wkT, "wvT": wvT, "woT": woT,
            "fc1T": fc1T, "fc2T": fc2T,
            "uT": uTf[:, :, c * VS : (c + 1) * VS].copy(),
            "masks": masks,
            "remw": np.full((P, 1), 1.0 if h == 1 else 0.0, np.float32),
            "sel4": sel4,
            "pairsel": np.array([[(c % 2) ^ 1]], np.int32),
            **lns,
        }
        in_maps.append(m)
    return in_maps


_CACHED = {}


def kernel(**inputs):
    from concourse.bass_utils import run_bass_kernel_spmd
    pidx = int(np.asarray(inputs["predict_idx"]))
    key = ("nc", pidx % T)
    if key not in _CACHED:
        _CACHED[key] = build_nc(pcol=pidx % T)
    nc = _CACHED[key]
    in_maps = prepare_inputs(**inputs)
    res = run_bass_kernel_spmd(nc, in_maps, core_ids=list(range(NC_)), trace=False)
    return np.concatenate([res.results[c]["out"] for c in range(NC_)], axis=1)


# revision 29
# speedup vs baseline: 1.0629x; 1.0386x over previous
"""MicroGPT forward pass on 8 Trainium2 NeuronCores (Bass/Tile).

Sharding: token-sharded — core c = 2*b + h owns batch b, sequence half h
(512 contiguous tokens). Activations are feature-major in SBUF
(x^T: [768 rows -> 6 tiles of 128, 512 token cols]); all matmuls fp32r/f16.
Attention: S^T = K^T-slice (stationary) x Q^T (moving); softmax without max
subtraction (scores bounded); denominators via a ones column appended to V.
K/V slots 0-3 are the core's own 4 blocks (block-causal masks, identical on
every core); slots 4-7 are the pair core's blocks, weighted by a per-core
0/1 scalar (1 when the pair holds earlier positions). Per layer a PAIRWISE
(2-rank) AllGather shares K^T/V; each core reads its pair's shard with a
register-indexed dynamic DMA. All per-token statistic broadcasts (LN
rstd/mean, softmax reciprocal) are done with K=1 matmuls on the PE instead
of DMA round-trips. Gelu is a single fused ACT op; the FFN runs fc1 for all
24 f-tiles first, then fc2 m-outer with all fc2 weights SBUF-resident so
the PE streams uninterrupted. Final token: masked AllReduce, then final LN +
vocab-sharded unembed (4000 vocab rows per core, preloaded to SBUF).
"""
import sys, math

sys.path.insert(0, "/opt/trn_rl_repo")
import numpy as np

import concourse.bass as bass
import concourse.bacc as bacc
import concourse.mybir as mybir
import concourse.tile as tile

D, NH, DH, FF, NL, V = 768, 12, 64, 3072, 4, 32000
B, S = 4, 1024
EPS = 1e-5
NC_ = 8
P = 128
T = 512            # tokens per core
DT = D // P        # 6 d-model tiles
FT = FF // P       # 24 ff tiles
KB = 8             # key slots (0-3 own, 4-7 pair)
VS = V // NC_      # 4000 vocab rows per core
VCH = 8            # vocab chunks of 500
VCW = VS // VCH    # 500
F32 = mybir.dt.float32
F32R = mybir.dt.float32r
F16 = mybir.dt.float16
BF16 = mybir.dt.bfloat16
I32 = mybir.dt.int32
AF = mybir.ActivationFunctionType
OP = mybir.AluOpType
SCALE = 1.0 / math.sqrt(DH)
VW = NH * (DH + 1)           # 780 — V tile width incl. ones cols
CONTRIB_W = DT * T + 4 * VW  # AllGather contribution width


# ---------------------------------------------------------------- bass program
def build_nc(n_layers=NL, pcol=511, dbg=False):
    nc = bacc.Bacc(None, target_bir_lowering=False, debug=False, num_devices=NC_)

    x0T = nc.dram_tensor("x0T", [DT, P, T], F32R, kind="ExternalInput")
    wqT = nc.dram_tensor("wqT", [n_layers, P, DT * D], F16, kind="ExternalInput")
    wkT = nc.dram_tensor("wkT", [n_layers, P, DT * D], F16, kind="ExternalInput")
    wvT = nc.dram_tensor("wvT", [n_layers, P, DT * D], F16, kind="ExternalInput")
    woT = nc.dram_tensor("woT", [n_layers, P, DT * D], F16, kind="ExternalInput")
    fc1T = nc.dram_tensor("fc1T", [n_layers, FT, P, DT * P], F16, kind="ExternalInput")
    fc2T = nc.dram_tensor("fc2T", [n_layers, FT, P, D], F16, kind="ExternalInput")
    ln1g = nc.dram_tensor("ln1g", [n_layers, P, DT], F32, kind="ExternalInput")
    ln1b = nc.dram_tensor("ln1b", [n_layers, P, DT], F32, kind="ExternalInput")
    ln2g = nc.dram_tensor("ln2g", [n_layers, P, DT], F32, kind="ExternalInput")
    ln2b = nc.dram_tensor("ln2b", [n_layers, P, DT], F32, kind="ExternalInput")
    lnfg = nc.dram_tensor("lnfg", [P, DT], F32, kind="ExternalInput")
    lnfb = nc.dram_tensor("lnfb", [P, DT], F32, kind="ExternalInput")
    uT = nc.dram_tensor("uT", [DT, P, VS], F16, kind="ExternalInput")
    masks = nc.dram_tensor("masks", [P, P], F16, kind="ExternalInput")
    remw = nc.dram_tensor("remw", [P, 1], F32, kind="ExternalInput")
    sel4 = nc.dram_tensor("sel4", [P, B], F32R, kind="ExternalInput")
    pairsel = nc.dram_tensor("pairsel", [1, 1], I32, kind="ExternalInput")

    out = nc.dram_tensor("out", [B, VS], F32, kind="ExternalOutput")

    from contextlib import ExitStack
    with tile.TileContext(nc) as tc:
        with ExitStack() as _stk:
            _p = lambda *a, **kw: _stk.enter_context(tc.tile_pool(*a, **kw))
            cpool = _p(name="const", bufs=1)
            ppool = _p(name="persist", bufs=1)
            xpool = _p(name="xp", bufs=6)
            hpool = _p(name="hp", bufs=6)
            qpool = _p(name="qp", bufs=6)
            apool = _p(name="ac", bufs=6)
            wpool = _p(name="wp", bufs=3)
            fpool = _p(name="fp", bufs=3)
            f2pool = _p(name="f2p", bufs=FT)
            gpool = _p(name="gp", bufs=FT)
            upool = _p(name="up", bufs=4)
            epool = _p(name="ep", bufs=4)
            spool = _p(name="sp", bufs=5)
            psr = _p(name="psr", bufs=2, space="PSUM")
            psa = _p(name="psa", bufs=2, space="PSUM")
            psb = _p(name="psb", bufs=2, space="PSUM")
            dpool = _p(name="dram", bufs=2, space="DRAM")
            # ---- constants (memset cannot write f32r; stage via f32 + copy)
            ones_f32 = cpool.tile([P, 1], F32)
            nc.vector.memset(ones_f32[:], 1.0)
            trimask = cpool.tile([P, P], F16)
            nc.sync.dma_start(trimask[:], masks[:])
            ones_col = cpool.tile([P, 1], F32R)
            nc.vector.tensor_copy(ones_col[:], ones_f32[:])
            onesr_f32 = cpool.tile([1, P], F32)
            nc.vector.memset(onesr_f32[:], 1.0)
            ones_row = cpool.tile([1, P], F32R)
            nc.vector.tensor_copy(ones_row[:], onesr_f32[:])
            eps1 = cpool.tile([1, 1], F32)
            nc.vector.memset(eps1[:], EPS)
            hse_f = cpool.tile([1, P], F32)
            nc.vector.memset(hse_f[:], 0.0)
            nc.vector.memset(hse_f[0:1, 0:DH], 1.0)
            hsel_e = cpool.tile([1, P], F32R)
            nc.vector.tensor_copy(hsel_e[:], hse_f[:])
            hso_f = cpool.tile([1, P], F32)
            nc.vector.memset(hso_f[:], 0.0)
            nc.vector.memset(hso_f[0:1, DH:P], 1.0)
            hsel_o = cpool.tile([1, P], F32R)
            nc.vector.tensor_copy(hsel_o[:], hso_f[:])
            sel4_sb = cpool.tile([P, B], F32R)
            nc.sync.dma_start(sel4_sb[:], sel4[:])
            remw_sb = cpool.tile([P, 1], F32)
            nc.sync.dma_start(remw_sb[:], remw[:])

            # persistent K^T / V buffers (slots 0-3 own, 4-7 pair)
            KT = [ppool.tile([P, KB * P], F16, tag=f"kt{e}", name=f"KT{e}")
                  for e in range(DT)]
            VT = [ppool.tile([P, VW], F16, tag=f"vt{j}", name=f"VT{j}")
                  for j in range(KB)]
            for j in range(4):
                for h in range(NH):
                    nc.vector.tensor_copy(
                        VT[j][:, h * (DH + 1) + DH : h * (DH + 1) + DH + 1],
                        ones_f32[:])

            # pair rank register for dynamic reads of the AllGather output
            with tc.tile_critical():
                with nc.sync.register("pairreg") as preg:
                    nc.sync.reg_load(preg, pairsel[0:1, 0:1])
                    pv = nc.sync.snap(preg, min_val=0, max_val=1)

            # ---- residual stream (updated in place by residual adds)
            xT = []
            for k in range(DT):
                t_ = xpool.tile([P, T], F32R, tag="xT", name=f"xT{k}")
                nc.sync.dma_start(t_[:], x0T[k])
                xT.append(t_)

            def layer_norm(g_dram, b_dram, l):
                """mean/var via PE stats matmuls; rstd & -mean*rstd broadcast
                to all partitions with a K=1 matmul (no DMA round-trip)."""
                gb = spool.tile([P, 2 * DT], F32, tag="lngb", bufs=3, name="gb")
                nc.sync.dma_start(gb[:, 0:DT], g_dram[l])
                nc.sync.dma_start(gb[:, DT : 2 * DT], b_dram[l])
                sum_ps = psa.tile([1, T], F32, tag="acc", space="PSUM", name="sum_ps")
                sq_ps = psa.tile([1, T], F32, tag="acc", space="PSUM", name="sq_ps")
                sum_ps, sq_ps = sum_ps[:], sq_ps[:]
                for k in range(DT):
                    xsq = epool.tile([P, T], F32R, tag="lntmp", name="xsq")
                    nc.vector.tensor_mul(xsq[:], xT[k][:], xT[k][:])
                    nc.tensor.matmul(sum_ps, ones_col[:], xT[k][:],
                                     start=(k == 0), stop=(k == DT - 1))
                    nc.tensor.matmul(sq_ps, ones_col[:], xsq[:],
                                     start=(k == 0), stop=(k == DT - 1))
                sums_sb = spool.tile([1, T], F32, tag="lnstat", bufs=4, name="sums_sb")
                nc.vector.tensor_copy(sums_sb[:], sum_ps)
                m2s = spool.tile([1, T], F32, tag="lnstat", bufs=4, name="m2s")
                nc.vector.scalar_tensor_tensor(out=m2s[:], in0=sums_sb[:],
                                               scalar=1.0 / (D * D), in1=sums_sb[:],
                                               op0=OP.mult, op1=OP.mult)
                var = spool.tile([1, T], F32, tag="lnstat", bufs=4, name="var")
                nc.vector.scalar_tensor_tensor(out=var[:], in0=sq_ps,
                                               scalar=1.0 / D, in1=m2s[:],
                                               op0=OP.mult, op1=OP.subtract)
                std = spool.tile([1, T], F32, tag="lnstat", bufs=4, name="std")
                nc.scalar.activation(std[:], var[:], AF.Sqrt, bias=eps1[:])
                # rm = [rstd | -mean*rstd] in one f32r row
                rstd = spool.tile([1, T], F32, tag="lnstat", bufs=4, name="rstd")
                nc.vector.reciprocal_approx_fast(out=rstd[:], in_=std[:])
                rm = spool.tile([1, 2 * T], F32R, tag="lnr", bufs=2, name="rm")
                nc.vector.tensor_copy(rm[0:1, 0:T], rstd[:])
                nc.vector.scalar_tensor_tensor(out=rm[0:1, T : 2 * T],
                                               in0=sums_sb[:], scalar=-1.0 / D,
                                               in1=rstd[:],
                                               op0=OP.mult, op1=OP.mult)
                bc_ps = psb.tile([P, 2 * T], F32, tag="bc", space="PSUM",
                                 name="bc_ps")
                nc.tensor.matmul(bc_ps[:, 0:T], ones_row[:],
                                 rm[0:1, 0:T], start=True, stop=True)
                nc.tensor.matmul(bc_ps[:, T : 2 * T], ones_row[:],
                                 rm[0:1, T : 2 * T], start=True, stop=True)
                hT = []
                for k in range(DT):
                    t1 = epool.tile([P, T], F32, tag="lntmp", name="lnt1")
                    nc.vector.tensor_mul(t1[:], bc_ps[:, 0:T], xT[k][:])
                    t2 = epool.tile([P, T], F32, tag="lntmp", name="lnt2")
                    nc.vector.tensor_add(t2[:], bc_ps[:, T : 2 * T], t1[:])
                    h_ = hpool.tile([P, T], F16, tag="hT", name="hT_t")
                    nc.scalar.activation(h_[:], t2[:], AF.Identity,
                                         scale=gb[:, k : k + 1],
                                         bias=gb[:, DT + k : DT + k + 1])
                    hT.append(h_)
                return hT

            for l in range(n_layers):
                with nc.named_scope(f"L{l}"):
                    hT = layer_norm(ln1g, ln1b, l)

                    # ---- K^T, V first (feeds AllGather early), then Q^T
                    wk_sb = wpool.tile([P, DT * D], F16, tag="w", name="wk_sb")
                    nc.sync.dma_start(wk_sb[:], wkT[l])
                    for m in range(DT):
                        k_ps = psr.tile([P, T], F32, tag="rot", space="PSUM", name="k_ps")
                        for k in range(DT):
                            nc.tensor.matmul(
                                k_ps[:], wk_sb[:, k * D + m * P : k * D + (m + 1) * P],
                                hT[k][:], start=(k == 0), stop=(k == DT - 1))
                        nc.vector.tensor_copy(KT[m][:, 0:T], k_ps[:])

                    wv_sb = wpool.tile([P, DT * D], F16, tag="w", name="wv_sb")
                    nc.sync.dma_start(wv_sb[:], wvT[l])
                    for m in range(4):
                        for c in range(2):
                            v_ps = psr.tile([P, 6 * DH], F32, tag="rot", space="PSUM",
                                            name="v_ps")
                            for k in range(DT):
                                nc.tensor.matmul(
                                    v_ps[:], hT[k][:, m * P : (m + 1) * P],
                                    wv_sb[:, k * D + c * 6 * DH : k * D + (c + 1) * 6 * DH],
                                    start=(k == 0), stop=(k == DT - 1))
                            dst = VT[m][:, c * 6 * (DH + 1) : (c + 1) * 6 * (DH + 1)] \
                                .rearrange("p (h e) -> p h e", h=6, e=DH + 1)[:, :, 0:DH]
                            src = v_ps[:].rearrange("p (h e) -> p h e", h=6, e=DH)
                            nc.vector.tensor_copy(dst, src)

                    # ---- share K^T/V with the pair core (2-rank AllGather)
                    contrib = dpool.tile([P, CONTRIB_W], F16, tag="contrib", name="contrib")
                    for e in range(DT):
                        nc.sync.dma_start(contrib[:, e * T : (e + 1) * T],
                                          KT[e][:, 0:T])
                    for m in range(4):
                        nc.sync.dma_start(
                            contrib[:, DT * T + m * VW : DT * T + (m + 1) * VW],
                            VT[m][:])
                    gout = dpool.tile([2, P, CONTRIB_W], F16, tag="gout",
                                      name="gout")
                    nc.gpsimd.collective_compute(
                        "AllGather", OP.bypass,
                        ins=[contrib[:].opt()],
                        outs=[gout[:].opt()],
                        replica_groups=[[2 * i, 2 * i + 1] for i in range(4)],
                    )
                    rsrc = gout[bass.ds(pv, 1)]
                    for e in range(DT):
                        nc.sync.dma_start(KT[e][:, T : 2 * T],
                                          rsrc[0, :, e * T : (e + 1) * T])
                    for m in range(4):
                        nc.sync.dma_start(
                            VT[4 + m][:],
                            rsrc[0, :, DT * T + m * VW : DT * T + (m + 1) * VW])
                        nc.vector.tensor_scalar_mul(VT[4 + m][:], VT[4 + m][:],
                                                    remw_sb[:, 0:1])

                    # prefetch fc2 weights during attention
                    f2ws = []
                    for f in range(FT):
                        f2w = f2pool.tile([P, D], F16, tag="f2w", name="f2w")
                        nc.sync.dma_start(f2w[:], fc2T[l, f])
                        f2ws.append(f2w)

                    wq_sb = wpool.tile([P, DT * D], F16, tag="w", name="wq_sb")
                    nc.sync.dma_start(wq_sb[:], wqT[l])
                    QT = []
                    for m in range(DT):
                        q_ps = psr.tile([P, T], F32, tag="rot", space="PSUM", name="q_ps")
                        for k in range(DT):
                            nc.tensor.matmul(
                                q_ps[:], wq_sb[:, k * D + m * P : k * D + (m + 1) * P],
                                hT[k][:], start=(k == 0), stop=(k == DT - 1))
                        qt = qpool.tile([P, T], F16, tag="qt", name="qt")
                        nc.vector.tensor_copy(qt[:], q_ps[:])
                        QT.append(qt)

                    # ---- attention, head pairs (2 heads share one attnC tile)
                    # own slots j<4: only queries >= slot start (suffix);
                    # remote slots: full width, merged j={4,5} / {6,7} into one
                    # [P, 2T] score tile -> single exp; V zeroed on h=0 cores.
                    attnC = [apool.tile([P, T], F16, tag="attnC", name=f"attnC{e}")
                             for e in range(DT)]
                    # phase 1: OWN slots for every pair (no AllGather dependency
                    # -> runs during the collective); partial numerators land in
                    # attnC, denominators at partitions 0/64 of a small tile.
                    denO = []
                    for e in range(DT):
                        h0, h1 = 2 * e, 2 * e + 1
                        attn_ps = {
                            h0: psa.tile([DH + 1, T], F32, tag="acc",
                                         space="PSUM", name=f"attnps{h0}"),
                            h1: psa.tile([DH + 1, T], F32, tag="acc",
                                         space="PSUM", name=f"attnps{h1}"),
                        }
                        # own slots merged into two score tiles per head:
                        # A = slots 0+1 ([P,896] on the wide bufs, idle here),
                        # B = slots 2+3 ([P,384]); one exp per tile.
                        swA, swB = {}, {}
                        for h in (h0, h1):
                            swA[h] = psb.tile([P, 2 * T], F32, tag="bc",
                                              space="PSUM", name="swA")
                            swB[h] = psr.tile([P, T], F32, tag="rot",
                                              space="PSUM", name="swB")
                        for j, lo, N in ((0, 0, T), (1, T, 384)):
                            for h in (h0, h1):
                                base = (h % 2) * DH
                                nc.tensor.matmul(
                                    swA[h][:, lo : lo + N],
                                    KT[e][base : base + DH, j * P : (j + 1) * P],
                                    QT[e][base : base + DH, j * P : T],
                                    start=True, stop=True)
                        for j, lo, N in ((2, 0, 256), (3, 256, 128)):
                            for h in (h0, h1):
                                base = (h % 2) * DH
                                nc.tensor.matmul(
                                    swB[h][:, lo : lo + N],
                                    KT[e][base : base + DH, j * P : (j + 1) * P],
                                    QT[e][base : base + DH, j * P : T],
                                    start=True, stop=True)
                        for h in (h0, h1):
                            eA = epool.tile([P, 2 * T], F16, tag="e", name="eA")
                            nc.scalar.activation(eA[:, 0 : T + 384],
                                                 swA[h][:, 0 : T + 384],
                                                 AF.Exp, scale=SCALE)
                            nc.vector.tensor_mul(eA[:, 0:P], eA[:, 0:P], trimask[:])
                            nc.vector.tensor_mul(eA[:, T : T + P], eA[:, T : T + P],
                                                 trimask[:])
                            eB = epool.tile([P, T], F16, tag="e", name="eB")
                            nc.scalar.activation(eB[:, 0:384], swB[h][:, 0:384],
                                                 AF.Exp, scale=SCALE)
                            nc.vector.tensor_mul(eB[:, 0:P], eB[:, 0:P], trimask[:])
                            nc.vector.tensor_mul(eB[:, 256:384], eB[:, 256:384],
                                                 trimask[:])
                            hc = slice(h * (DH + 1), (h + 1) * (DH + 1))
                            nc.tensor.matmul(attn_ps[h][:, 0:T], VT[0][:, hc],
                                             eA[:, 0:T], start=True, stop=False)
                            nc.tensor.matmul(attn_ps[h][:, P:T], VT[1][:, hc],
                                             eA[:, T : T + 384],
                                             start=False, stop=False)
                            nc.tensor.matmul(attn_ps[h][:, 2 * P : T], VT[2][:, hc],
                                             eB[:, 0:256], start=False, stop=False)
                            nc.tensor.matmul(attn_ps[h][:, 3 * P : T], VT[3][:, hc],
                                             eB[:, 256:384], start=False, stop=True)
                        dp = spool.tile([DH + 1, T], F16, tag="deno", bufs=6,
                                        name="dp")
                        for h in (h0, h1):
                            base = (h % 2) * DH
                            nc.vector.tensor_copy(attnC[e][base : base + DH, :],
                                                  attn_ps[h][0:DH, :])
                            nc.vector.tensor_copy(dp[base : base + 1, :],
                                                  attn_ps[h][DH : DH + 1, :])
                        denO.append(dp)
                    # phase 2: REMOTE slots (pair data) + combine + normalize
                    for e in range(DT):
                        h0, h1 = 2 * e, 2 * e + 1
                        attn_ps = {
                            h0: psa.tile([DH + 1, T], F32, tag="acc",
                                         space="PSUM", name=f"attnpr{h0}"),
                            h1: psa.tile([DH + 1, T], F32, tag="acc",
                                         space="PSUM", name=f"attnpr{h1}"),
                        }
                        for jp in (4, 6):
                            sws = {}
                            for h in (h0, h1):
                                sws[h] = psb.tile([P, 2 * T], F32, tag="bc",
                                                  space="PSUM", name="sw")
                            for jo in (0, 1):
                                for h in (h0, h1):
                                    base = (h % 2) * DH
                                    nc.tensor.matmul(
                                        sws[h][:, jo * T : (jo + 1) * T],
                                        KT[e][base : base + DH,
                                              (jp + jo) * P : (jp + jo + 1) * P],
                                        QT[e][base : base + DH, :],
                                        start=True, stop=True)
                            ews = {}
                            for h in (h0, h1):
                                ew = epool.tile([P, 2 * T], F16, tag="e", name="ew")
                                nc.scalar.activation(ew[:], sws[h][:], AF.Exp,
                                                     scale=SCALE)
                                ews[h] = ew
                            for h in (h0, h1):
                                nc.tensor.matmul(
                                    attn_ps[h][:],
                                    VT[jp][:, h * (DH + 1) : (h + 1) * (DH + 1)],
                                    ews[h][:, 0:T],
                                    start=(jp == 4), stop=False)
                                nc.tensor.matmul(
                                    attn_ps[h][:],
                                    VT[jp + 1][:, h * (DH + 1) : (h + 1) * (DH + 1)],
                                    ews[h][:, T : 2 * T],
                                    start=False, stop=(jp == 6))
                        nrm_ps = psr.tile([P, T], F32, tag="rot", space="PSUM",
                                          name="nrm_ps")
                        for h in (h0, h1):
                            base = (h % 2) * DH
                            ssb = spool.tile([1, T], F32, tag="recip", bufs=4, name="ssb")
                            nc.vector.tensor_add(ssb[:], attn_ps[h][DH : DH + 1, :],
                                                 denO[e][base : base + 1, :])
                            rr = spool.tile([1, T], F32, tag="recip", bufs=4, name="rr")
                            nc.vector.reciprocal_approx_fast(out=rr[:], in_=ssb[:])
                            rrr = spool.tile([1, T], F32R, tag="recip", bufs=4, name="rrr")
                            nc.vector.tensor_copy(rrr[:], rr[:])
                            hsel = hsel_e if h == h0 else hsel_o
                            nc.tensor.matmul(nrm_ps[:], hsel[:], rrr[:],
                                             start=(h == h0), stop=(h == h1))
                        nrm_sb = epool.tile([P, T], F32, tag="lntmp", name="nrm_sb")
                        nc.vector.tensor_copy(nrm_sb[:], nrm_ps[:])
                        for h in (h0, h1):
                            base = (h % 2) * DH
                            nc.vector.tensor_add(attnC[e][base : base + DH, :],
                                                 attn_ps[h][0:DH, :],
                                                 attnC[e][base : base + DH, :])
                            nc.vector.tensor_mul(attnC[e][base : base + DH, :],
                                                 attnC[e][base : base + DH, :],
                                                 nrm_sb[base : base + DH, :])

                    # ---- output projection + residual (in place)
                    wo_sb = wpool.tile([P, DT * D], F16, tag="w", name="wo_sb")
                    nc.sync.dma_start(wo_sb[:], woT[l])
                    for m in range(DT):
                        o_ps = psr.tile([P, T], F32, tag="rot", space="PSUM", name="o_ps")
                        for k in range(DT):
                            nc.tensor.matmul(
                                o_ps[:], wo_sb[:, k * D + m * P : k * D + (m + 1) * P],
                                attnC[k][:], start=(k == 0), stop=(k == DT - 1))
                        nc.vector.tensor_add(xT[m][:], o_ps[:], xT[m][:])

                    # ---- FFN: fc1+gelu for all f-tiles, then fc2 m-outer
                    h2T = layer_norm(ln2g, ln2b, l)
                    gT = []
                    for f in range(FT):
                        f1w = fpool.tile([P, DT * P], F16, tag="f1w", name="f1w")
                        nc.sync.dma_start(f1w[:], fc1T[l, f])
                        f1_ps = psr.tile([P, T], F32, tag="rot", space="PSUM", name="f1_ps")
                        for k in range(DT):
                            nc.tensor.matmul(f1_ps[:], f1w[:, k * P : (k + 1) * P],
                                             h2T[k][:], start=(k == 0), stop=(k == DT - 1))
                        g_sb = gpool.tile([P, T], F16, tag="g", name="g_sb")
                        nc.scalar.activation(g_sb[:], f1_ps[:], AF.Gelu_apprx_tanh)
                        gT.append(g_sb)
                    for m in range(DT):
                        x2_ps = psa.tile([P, T], F32, tag="acc", space="PSUM",
                                         name="x2_ps")
                        for f in range(FT):
                            nc.tensor.matmul(x2_ps[:], f2ws[f][:, m * P : (m + 1) * P],
                                             gT[f][:], start=(f == 0), stop=(f == FT - 1))
                        nc.vector.tensor_add(xT[m][:], x2_ps[:], xT[m][:])

            # unembed streaming: issue all chunk loads now so they fill DMA
            # queues during the last layer; consumed at the very end
            uSB = {}
            for ci in range(VCH):
                for k in range(DT):
                    u_sb = upool.tile([P, VCW], F16, tag="us", name="u_sb")
                    nc.sync.dma_start(u_sb[:], uT[k, :, ci * VCW : (ci + 1) * VCW])
                    uSB[(ci, k)] = u_sb

            # ---- final: masked AllReduce of predicted token's x column
            with nc.named_scope("final"):
                cont = dpool.tile([P, DT * B], F32, tag="cont", name="cont")
                csb = spool.tile([P, DT * B], F32, tag="csb", name="csb")
                for k in range(DT):
                    nc.vector.tensor_mul(
                        csb[:, k * B : (k + 1) * B],
                        xT[k][:, pcol : pcol + 1].to_broadcast((P, B)),
                        sel4_sb[:])
                nc.sync.dma_start(cont[:], csb[:])
                ar_out = dpool.tile([P, DT * B], F32, tag="arout",
                                    addr_space="Shared", name="ar_out")
                nc.gpsimd.collective_compute(
                    "AllReduce", OP.add,
                    ins=[cont[:].opt()],
                    outs=[ar_out[:].opt()],
                    replica_groups=[list(range(NC_))],
                )
                xf_raw = spool.tile([P, DT * B], F32, tag="xfraw", name="xf_raw")
                nc.sync.dma_start(xf_raw[:], ar_out[:])
                xf = spool.tile([P, DT * B], F32R, tag="xf", name="xf")
                nc.vector.tensor_copy(xf[:], xf_raw[:])

                lgb = spool.tile([P, 2 * DT], F32, tag="lngb", bufs=3, name="lgb")
                nc.sync.dma_start(lgb[:, 0:DT], lnfg[:])
                nc.sync.dma_start(lgb[:, DT : 2 * DT], lnfb[:])
                fs_ps = psa.tile([1, B], F32, tag="acc", space="PSUM", name="fs_ps")
                fq_ps = psa.tile([1, B], F32, tag="acc", space="PSUM", name="fq_ps")
                xfsq = spool.tile([P, DT * B], F32R, tag="xfsq", name="xfsq")
                nc.vector.tensor_mul(xfsq[:], xf[:], xf[:])
                for k in range(DT):
                    nc.tensor.matmul(fs_ps[:], ones_col[:], xf[:, k * B : (k + 1) * B],
                                     start=(k == 0), stop=(k == DT - 1))
                    nc.tensor.matmul(fq_ps[:], ones_col[:], xfsq[:, k * B : (k + 1) * B],
                                     start=(k == 0), stop=(k == DT - 1))
                fmean = spool.tile([1, B], F32, tag="lnstat", bufs=4, name="fmean")
                nc.vector.tensor_scalar_mul(fmean[:], fs_ps[:], 1.0 / D)
                fm2 = spool.tile([1, B], F32, tag="lnstat", bufs=4, name="fm2")
                nc.vector.tensor_mul(fm2[:], fmean[:], fmean[:])
                fsqd = spool.tile([1, B], F32, tag="lnstat", bufs=4, name="fsqd")
                nc.vector.tensor_scalar_mul(fsqd[:], fq_ps[:], 1.0 / D)
                fvar = spool.tile([1, B], F32, tag="lnstat", bufs=4, name="fvar")
                nc.vector.tensor_sub(fvar[:], fsqd[:], fm2[:])
                fstd = spool.tile([1, B], F32, tag="lnstat", bufs=4, name="fstd")
                nc.scalar.activation(fstd[:], fvar[:], AF.Sqrt, bias=eps1[:])
                # frm = [rstd | -mean*rstd]
                frstd = spool.tile([1, B], F32, tag="lnstat", bufs=4, name="frstd")
                nc.vector.reciprocal(frstd[:], fstd[:])
                frm = spool.tile([1, 2 * B], F32R, tag="lnr", bufs=2, name="frm")
                nc.vector.tensor_copy(frm[0:1, 0:B], frstd[:])
                nc.vector.scalar_tensor_tensor(out=frm[0:1, B : 2 * B],
                                               in0=fmean[:], scalar=-1.0,
                                               in1=frstd[:],
                                               op0=OP.mult, op1=OP.mult)
                fbc_ps = psr.tile([P, 2 * B], F32, tag="rot", space="PSUM",
                                  name="fbc_ps")
                nc.tensor.matmul(fbc_ps[:], ones_row[:], frm[:],
                                 start=True, stop=True)
                xfn = spool.tile([P, DT * B], F16, tag="xfn", name="xfn")
                for k in range(DT):
                    t1 = spool.tile([P, B], F32, tag="lnstat", bufs=4, name="ft1")
                    nc.vector.tensor_mul(t1[:], fbc_ps[:, 0:B],
                                         xf[:, k * B : (k + 1) * B])
                    t2 = spool.tile([P, B], F32, tag="lnstat", bufs=4, name="ft2")
                    nc.vector.tensor_add(t2[:], fbc_ps[:, B : 2 * B], t1[:])
                    nc.scalar.activation(xfn[:, k * B : (k + 1) * B], t2[:], AF.Identity,
                                         scale=lgb[:, k : k + 1],
                                         bias=lgb[:, DT + k : DT + k + 1])

                for ci in range(VCH):
                    lg_ps = psr.tile([B, VCW], F32, tag="rot", space="PSUM", name="lg_ps")
                    for k in range(DT):
                        nc.tensor.matmul(lg_ps[:], xfn[:, k * B : (k + 1) * B],
                                         uSB[(ci, k)][:],
                                         start=(k == 0), stop=(k == DT - 1))
                    och = fpool.tile([B, VCW], F32, tag="och", name="och")
                    nc.vector.tensor_copy(och[:], lg_ps[:])
                    nc.sync.dma_start(out[:, ci * VCW : (ci + 1) * VCW], och[:])

    nc.compile()
    return nc


# ---------------------------------------------------------------- host side
def _positional_encoding(s, d):
    idx = np.arange(d)
    exponent = ((2 * (idx // 2)).astype(np.float32) / float(d)).astype(np.float32)
    pos = np.arange(s, dtype=np.float32)[:, None]
    angle = pos / np.power(np.float32(10000.0), exponent[None, :], dtype=np.float32)
    return np.where((idx % 2 == 0)[None, :], np.sin(angle), np.cos(angle)).astype(np.float32)


def _build_masks():
    """trimask[r, c] = 1 if key r <= query c (within-block causal)."""
    r = np.arange(P)
    return (r[:, None] <= r[None, :]).astype(np.float16)


def prepare_inputs(tokens, predict_idx, embedding, ln1_g, ln1_b, wq, wk, wv, wo,
                   ln2_g, ln2_b, fc1, fc2, lnf_g, lnf_b, unembed, n_layers=NL):
    f = lambda a: np.ascontiguousarray(np.asarray(a), dtype=np.float32)
    tokens = np.asarray(tokens)
    emb = f(embedding)
    pos = _positional_encoding(S, D)

    def wlayout(a):  # [L, out, in] -> [L, P, DT*D] with [l, p, k*D + dout]
        aT = a.transpose(0, 2, 1)
        return np.ascontiguousarray(
            aT.reshape(n_layers, DT, P, D).transpose(0, 2, 1, 3)
            .reshape(n_layers, P, DT * D)).astype(np.float16)

    wqT = wlayout(f(wq)[:n_layers].reshape(-1, NH * DH, D))
    wkT = wlayout(f(wk)[:n_layers].reshape(-1, NH * DH, D))
    wvT = wlayout(f(wv)[:n_layers].reshape(-1, NH * DH, D))
    woT = wlayout(f(wo)[:n_layers])
    fc1T = np.ascontiguousarray(
        f(fc1)[:n_layers].transpose(0, 2, 1)
        .reshape(n_layers, DT, P, FT, P).transpose(0, 3, 2, 1, 4)
        .reshape(n_layers, FT, P, DT * P)).astype(np.float16)
    fc2T = np.ascontiguousarray(
        f(fc2)[:n_layers].transpose(0, 2, 1)
        .reshape(n_layers, FT, P, D)).astype(np.float16)
    uTf = np.ascontiguousarray(f(unembed).T.reshape(DT, P, V)).astype(np.float16)

    def lnshape(a):
        return np.ascontiguousarray(
            f(a)[:n_layers].reshape(n_layers, DT, P).transpose(0, 2, 1))

    lns = {
        "ln1g": lnshape(ln1_g), "ln1b": lnshape(ln1_b),
        "ln2g": lnshape(ln2_g), "ln2b": lnshape(ln2_b),
        "lnfg": np.ascontiguousarray(f(lnf_g).reshape(DT, P).T),
        "lnfb": np.ascontiguousarray(f(lnf_b).reshape(DT, P).T),
    }
    masks = _build_masks()

    pidx = int(predict_idx)
    in_maps = []
    for c in range(NC_):
        b, h = c // 2, c % 2
        toks = np.asarray(tokens[b, h * T : (h + 1) * T]).astype(np.int64)
        x0 = emb.T[toks] + pos[h * T : (h + 1) * T]
        x0T = np.ascontiguousarray(x0.T.reshape(DT, P, T)).astype(np.float32)
        sel4 = np.zeros((P, B), np.float32)
        if pidx // T == h:
            sel4[:, b] = 1.0
        m = {
            "x0T": x0T, "wqT": wqT, "wkT": wkT, "wvT": wvT, "woT": woT,
            "fc1T": fc1T, "fc2T": fc2T,
            "uT": uTf[:, :, c * VS : (c + 1) * VS].copy(),
            "masks": masks,
            "remw": np.full((P, 1), 1.0 if h == 1 else 0.0, np.float32),
            "sel4": sel4,
            "pairsel": np.array([[(c % 2) ^ 1]], np.int32),
            **lns,
        }
        in_maps.append(m)
    return in_maps


_CACHED = {}


def kernel(**inputs):
    from concourse.bass_utils import run_bass_kernel_spmd
    pidx = int(np.asarray(inputs["predict_idx"]))
    key = ("nc", pidx % T)
    if key not in _CACHED:
        _CACHED[key] = build_nc(pcol=pidx % T)
    nc = _CACHED[key]
    in_maps = prepare_inputs(**inputs)
    res = run_bass_kernel_spmd(nc, in_maps, core_ids=list(range(NC_)), trace=False)
    return np.concatenate([res.results[c]["out"] for c in range(NC_)], axis=1)
